# revision 71
# baseline (speedup 1.0000x reference)
"""Gemma sliding-window attention layer on 8 Trainium2 NeuronCores.

Sharding: data-parallel over batch (B=2) x tensor-parallel over heads
(4 groups: 2 q heads + 1 kv head each) = 8 cores. Each core computes a
partial o-proj output [D, S] in bf16; host sums the 4 TP partials per
batch in fp32 and transposes back to [S, D].

Schedule (per 512-wide seq block b): proj(b) -> o-proj(b-1) -> attn(b),
so the rmsnorm/rope chain of block b and the softmax normalize of block
b-1 hide behind independent PE work. Within attention, score matmuls
run a fixed lookahead ahead of the PV matmuls so exp/mask latency never
stalls the PE. RMSNorm rinv = exp(-0.5*ln(ms)) keeps the activation
engine on a single function table (ln+exp share one set; sqrt doesn't).
RoPE runs in bf16 on SBUF for the DVE 2x mode. Half-width diagonal /
window-edge score tiles write into persistent half-zeroed pt tiles so
the PV accumulation stays full-width.
"""

import sys

sys.path.insert(0, "/opt/trn_rl_repo")

import math
from contextlib import ExitStack

import numpy as np
import ml_dtypes

import concourse.bass as bass
import concourse.bass_isa as bass_isa
import concourse.tile as tile
from concourse import bacc, mybir
from concourse.bass import ds, ts
from concourse.bass_utils import run_bass_kernel_spmd

BF16 = mybir.dt.bfloat16
F32 = mybir.dt.float32
NPBF16 = ml_dtypes.bfloat16
AF = mybir.ActivationFunctionType

H, KVH, DH, SW = 8, 4, 256, 1024
B, S, D = 2, 2048, 2048
EPS = 1e-6
ROPE_BASE = 10000.0
P = 128
SB = 512          # s-block width
NSB = S // SB     # 4
NDC = D // P      # 16 contraction chunks
EQ = 2 * DH       # per-core q width (2 heads)
QB = 256          # attention q-tile width
KBIAS = -math.log(16.0)  # folds the 1/sqrt(256) attn scale into k's rinv
LOOK = 3          # score-tile lookahead in the attention pipeline


def _kchunks(q0):
    """k-chunks for q-tile [q0, q0+QB).

    Returns (kc, mi, qoff, qw): k-chunk index, mask index or None, live
    q-column offset/width.  pT tile [k=128 (i), q (j)]: kpos = 128*kc + i,
    qpos = q0 + qoff + j; valid iff 0 <= qpos - kpos < SW.
    Masks: 0 -> j >= i (causal diag), 4 -> j <= i-1, 5 -> j <= i+127.
    d = 128*kc - q0:  d = 0 full mask0; d = +128 half tile (qoff 128,
    mask0 cols 0:128); d + SW = 0 half tile (qoff 0, mask4 cols 0:128);
    d + SW = 128 full tile mask5; other in-range chunks are mask-free.
    """
    res = []
    for KC in range(max(0, q0 - SW), q0 + QB, P):
        d = KC - q0
        if d == P:
            res.append((KC // P, 0, P, P))
        elif d == 0:
            res.append((KC // P, 0, 0, QB))
        elif d + SW == 0:
            res.append((KC // P, 4, 0, P))
        elif d + SW == P:
            res.append((KC // P, 5, 0, QB))
        else:
            res.append((KC // P, None, 0, QB))
    return res


def _build():
    nc = bacc.Bacc("TRN2", target_bir_lowering=False, debug=False)

    xT = nc.dram_tensor("xT", [D, S], BF16, kind="ExternalInput")
    wqT = nc.dram_tensor("wqT", [D, EQ], BF16, kind="ExternalInput")
    wkT = nc.dram_tensor("wkT", [D, DH], BF16, kind="ExternalInput")
    wvT = nc.dram_tensor("wvT", [D, DH], BF16, kind="ExternalInput")
    woT = nc.dram_tensor("woT", [EQ, D], BF16, kind="ExternalInput")
    trig = nc.dram_tensor("trig", [2, P, S], BF16, kind="ExternalInput")  # cos, sin
    masks = nc.dram_tensor("masks", [8, P, SB], BF16, kind="ExternalInput")
    onesd = nc.dram_tensor("onesd", [P, 1], BF16, kind="ExternalInput")
    out = nc.dram_tensor("out", [D, S], BF16, kind="ExternalOutput")

    # const tiles for activation bias operands
    for val in (EPS, KBIAS):
        t = nc.alloc_sbuf_tensor(f"const-bias-{val}", [P, 1], F32)
        nc.gpsimd.memset(t.ap(), val)
        nc.const_aps.aps[(F32, val)] = t.ap()
    nc.all_engine_barrier()

    with tile.TileContext(nc) as tc, ExitStack() as ctx:
        sp = ctx.enter_context(tc.tile_pool(name="sp", bufs=2))    # SBUF
        pp = ctx.enter_context(tc.tile_pool(name="pp", bufs=2, space="PSUM"))

        # ---- persistent SBUF tiles ----
        wq_sb = sp.tile([P, NDC * EQ], BF16, name="wq", tag="wq", bufs=1)
        wk_sb = sp.tile([P, NDC * DH], BF16, name="wk", tag="wk", bufs=1)
        wv_sb = sp.tile([P, NDC * DH], BF16, name="wv", tag="wv", bufs=1)
        wo_sb = sp.tile([P, 4 * D], BF16, name="wo", tag="wo", bufs=1)
        msk_sb = sp.tile([P, 8 * SB], BF16, name="msk", tag="msk", bufs=1)
        q_sb = [sp.tile([P, S], BF16, name=f"qsb{i}", tag="qsb", bufs=4) for i in range(4)]
        k_sb = [sp.tile([P, S], BF16, name=f"ksb{i}", tag="ksb", bufs=2) for i in range(2)]
        v_sb = [sp.tile([P, DH], BF16, name=f"vsb{i}", tag="vsb", bufs=NDC) for i in range(NDC)]
        ao_sb = [sp.tile([P, S], BF16, name=f"aosb{i}", tag="aosb", bufs=4) for i in range(4)]


        # ---- startup loads: k-ent weights + x block 0 first, fine-grained ----
        xT_r = xT.rearrange("(c p) s -> p c s", p=P)
        wk_r = wkT.rearrange("(c p) e -> p c e", p=P)
        wv_r = wvT.rearrange("(c p) e -> p c e", p=P)
        wq_r = wqT.rearrange("(c p) e -> p c e", p=P)

        xt0 = sp.tile([P, NDC * SB], BF16, name="xt0", tag="xt", bufs=2)
        nc.sync.dma_start(
            wk_sb[:, ds(0, 4 * DH)].rearrange("p (c e) -> p c e", c=4),
            wk_r[:, ds(0, 4), :])
        nc.sync.dma_start(
            xt0[:, ds(0, SB)].rearrange("p (c s) -> p c s", c=1),
            xT_r[:, ds(0, 1), ds(0, SB)])
        nc.sync.dma_start(
            xt0[:, ds(SB, SB)].rearrange("p (c s) -> p c s", c=1),
            xT_r[:, ds(1, 1), ds(0, SB)])
        for g in range(1, 8):
            nc.sync.dma_start(
                xt0[:, ds(g * 2 * SB, 2 * SB)].rearrange("p (c s) -> p c s", c=2),
                xT_r[:, ds(2 * g, 2), ds(0, SB)])
            if g in (1, 3, 5):
                gk = (g + 1) // 2
                nc.sync.dma_start(
                    wk_sb[:, ds(gk * 4 * DH, 4 * DH)].rearrange("p (c e) -> p c e", c=4),
                    wk_r[:, ds(4 * gk, 4), :])
        for g in range(4):
            nc.sync.dma_start(
                wq_sb[:, ds(g * 4 * EQ, 4 * EQ)].rearrange("p (c e) -> p c e", c=4),
                wq_r[:, ds(4 * g, 4), :])
        for g in range(2):
            nc.sync.dma_start(
                wv_sb[:, ds(g * 8 * DH, 8 * DH)].rearrange("p (c e) -> p c e", c=8),
                wv_r[:, ds(8 * g, 8), :])
        tg0 = sp.tile([P, 2 * SB], BF16, name="tg0", tag="tg", bufs=2)
        nc.sync.dma_start(tg0[:].rearrange("p (r s) -> p r s", r=2),
                          trig.rearrange("r p s -> p r s")[:, :, ds(0, SB)])

        def wq_ap(dc, eoff):
            return wq_sb[:, ds(dc * EQ + eoff, P)]

        def wk_ap(dc, eoff):
            return wk_sb[:, ds(dc * DH + eoff, P)]

        out_r = out.rearrange("(g p) s -> p g s", p=P)

        xts = {0: xt0}
        tgs = {0: tg0}

        # =================== phase emitters ===================

        def p1(blk):
            """Projections + rmsnorm + rope for seq block blk (k, V, q0, q1)."""
            xt = xts.pop(blk)
            tgt = tgs.pop(blk)
            sblice = ds(blk * SB, SB)
            tcos, tsin = tgt[:, 0:SB], tgt[:, SB : 2 * SB]

            ents = [("k", wk_ap, 0, 0), ("q0", wq_ap, 0, 32), ("q1", wq_ap, DH, 64)]

            def proj(ent, w_ap, eoff):
                pa = pp.tile([P, SB], F32, name=f"pa_{blk}_{ent}", tag="pj", bufs=2)
                pb = pp.tile([P, SB], F32, name=f"pb_{blk}_{ent}", tag="pj", bufs=2)
                for dc in range(NDC):
                    nc.tensor.matmul(pa[:], w_ap(dc, eoff), xt[:, ds(dc * SB, SB)],
                                     start=(dc == 0), stop=(dc == NDC - 1))
                pab = sp.tile([P, SB], BF16, name=f"pab_{blk}_{ent}", tag="pr", bufs=6)
                nc.vector.tensor_copy(pab[:], pa[:])
                for dc in range(NDC):
                    nc.tensor.matmul(pb[:], w_ap(dc, eoff + P), xt[:, ds(dc * SB, SB)],
                                     start=(dc == 0), stop=(dc == NDC - 1))
                pbb = sp.tile([P, SB], BF16, name=f"pbb_{blk}_{ent}", tag="pr", bufs=6)
                nc.vector.tensor_copy(pbb[:], pb[:])
                sqa = sp.tile([P, SB], BF16, name=f"sqa_{blk}_{ent}", tag="sq", bufs=4)
                sqb = sp.tile([P, SB], BF16, name=f"sqb_{blk}_{ent}", tag="sq", bufs=4)
                nc.vector.tensor_mul(sqa[:], pab[:], pab[:])
                nc.vector.tensor_mul(sqb[:], pbb[:], pbb[:])
                return pab, pbb, sqa, sqb

            def emit_sums(ent, sqa, sqb):
                # partition sums on the (otherwise idle) gpsimd engine; the
                # all-reduce result arrives already broadcast over partitions,
                # so ln/exp produce the full [P, SB] rinv with no bcast step.
                ra = sp.tile([P, SB], F32, name=f"ra_{blk}_{ent}", tag="ra", bufs=2)
                rb = sp.tile([P, SB], F32, name=f"rb_{blk}_{ent}", tag="ra", bufs=2)
                nc.gpsimd.partition_all_reduce(ra[:], sqa[:], P, bass_isa.ReduceOp.add)
                nc.gpsimd.partition_all_reduce(rb[:], sqb[:], P, bass_isa.ReduceOp.add)
                rs = sp.tile([P, SB], F32, name=f"rs_{blk}_{ent}", tag="rs", bufs=3)
                nc.gpsimd.tensor_add(rs[:], ra[:], rb[:])
                return rs

            def emit_ln(ent, rs):
                tln = sp.tile([P, SB], F32, name=f"tln_{blk}_{ent}", tag="tl", bufs=3)
                nc.scalar.activation(tln[:], rs[:], AF.Ln, bias=EPS, scale=1.0 / DH)
                return tln

            def emit_exp(ent, tln):
                qb = sp.tile([P, SB], BF16, name=f"qb_{blk}_{ent}", tag="qb", bufs=4)
                nc.scalar.activation(qb[:], tln[:], AF.Exp, scale=-0.5,
                                     bias=(KBIAS if ent == "k" else 0.0))
                return qb

            def emit_ropemuls(ent, qb, pab, pbb, lo, hi):
                ta = sp.tile([P, SB], BF16, name=f"ta_{blk}_{ent}", tag="rt", bufs=8)
                tb = sp.tile([P, SB], BF16, name=f"tb_{blk}_{ent}", tag="rt", bufs=8)
                tc_ = sp.tile([P, SB], BF16, name=f"tc_{blk}_{ent}", tag="rt", bufs=8)
                nc.vector.tensor_mul(ta[:], pab[:], tcos)
                nc.vector.tensor_mul(tb[:], pbb[:], tsin)
                nc.vector.tensor_sub(tc_[:], ta[:], tb[:])
                nc.vector.tensor_mul(lo[:, sblice], tc_[:], qb[:])
                td = sp.tile([P, SB], BF16, name=f"td_{blk}_{ent}", tag="rt", bufs=8)
                te = sp.tile([P, SB], BF16, name=f"te_{blk}_{ent}", tag="rt", bufs=8)
                tf = sp.tile([P, SB], BF16, name=f"tf_{blk}_{ent}", tag="rt", bufs=8)
                nc.vector.tensor_mul(td[:], pbb[:], tcos)
                nc.vector.tensor_mul(te[:], pab[:], tsin)
                nc.vector.tensor_add(tf[:], td[:], te[:])
                nc.vector.tensor_mul(hi[:, sblice], tf[:], qb[:])

            dests = {"k": (k_sb[0], k_sb[1]), "q0": (q_sb[0], q_sb[1]),
                     "q1": (q_sb[2], q_sb[3])}
            info = {}

            def chain_act(e):
                return emit_exp(e, emit_ln(e, emit_sums(e, info[e][2], info[e][3])))

            # --- k, q0, q1 projections, then V. Each ent's sums/ln/exp chain
            # goes out as soon as its squares exist (no act-table switches:
            # the table set is pinned, see compile below). Rope muls (DVE) are
            # emitted after the V copies so the pv/op psum rings never wait
            # behind them in the DVE queue; block 0 (no o-proj cover ahead of
            # attention) emits k/q0 ropes early instead.
            qbs = {}
            info["k"] = proj("k", wk_ap, 0)
            info["q0"] = proj("q0", wq_ap, 0)
            qbs["k"] = chain_act("k")
            if blk == 0:
                emit_ropemuls("k", qbs["k"], info["k"][0], info["k"][1], *dests["k"])
            info["q1"] = proj("q1", wq_ap, DH)
            qbs["q0"] = chain_act("q0")
            if blk == 0:
                emit_ropemuls("q0", qbs["q0"], info["q0"][0], info["q0"][1],
                              *dests["q0"])

            for sc in range(SB // P):
                pv = pp.tile([P, DH], F32, name=f"pv_{blk}_{sc}", tag="sc", bufs=3)
                for dc in range(NDC):
                    nc.tensor.matmul(pv[:], xt[:, ds(dc * SB + sc * P, P)],
                                     wv_sb[:, ds(dc * DH, DH)],
                                     start=(dc == 0), stop=(dc == NDC - 1))
                if sc == 0:
                    qbs["q1"] = chain_act("q1")
                nc.vector.tensor_copy(v_sb[blk * (SB // P) + sc][:], pv[:])
            late = ("q1",) if blk == 0 else ("k", "q0", "q1")
            for e in late:
                emit_ropemuls(e, qbs[e], info[e][0], info[e][1], *dests[e])

        def p3(blk):
            """o-projection for block blk -> DRAM. All copies on Act: its
            queue is clear here (norm batch finished during V), while DVE
            holds the rope batch. DMA per 4-op group keeps HWDGE slots few."""
            sblice = ds(blk * SB, SB)
            finegrain = blk == NSB - 1  # last block: per-op DMA, short drain
            for g4 in range(4):
                ob4 = sp.tile([P, 4 * SB], BF16, name=f"ob_{blk}_{g4}", tag="ob", bufs=2)
                for j in range(4):
                    dmc = 4 * g4 + j
                    op = pp.tile([P, SB], F32, name=f"op_{blk}_{dmc}", tag="pj", bufs=2)
                    for ec in range(4):
                        nc.tensor.matmul(op[:], wo_sb[:, ds(ec * D + dmc * P, P)],
                                         ao_sb[ec][:, sblice],
                                         start=(ec == 0), stop=(ec == 3))
                    if finegrain and g4 == 3 and j == 3:
                        nc.vector.tensor_copy(ob4[:, ds(j * SB, SB)], op[:])
                    else:
                        nc.scalar.copy(ob4[:, ds(j * SB, SB)], op[:])
                    if finegrain:
                        nc.sync.dma_start(
                            out_r[:, ds(dmc, 1), sblice],
                            ob4[:, ds(j * SB, SB)].rearrange("p (g s) -> p g s", g=1))
                if not finegrain:
                    nc.sync.dma_start(
                        out_r[:, ds(4 * g4, 4), sblice],
                        ob4[:].rearrange("p (g s) -> p g s", g=4))

        def p2(blk):
            """Attention for the two q-tiles of block blk.

            Chunk emission order sandwiches the half-width (trimmed) chunks
            between full ones, so the PV psum accumulation starts on the
            first full chunk (full width) and stops on the last full chunk,
            with the trims accumulating into column sub-ranges in between.
            q0=0 has a single full chunk, so its diagonal trim keeps the
            legacy zero-half pth tile and accumulates at full width.
            """
            for it, (sub, h) in enumerate([(s_, h_) for h_ in range(2) for s_ in range(2)]):
                q0 = blk * SB + sub * QB
                raw = _kchunks(q0)
                fulls = [c for c in raw if c[3] == QB]
                trims = [c for c in raw if c[3] != QB]
                if len(fulls) >= 2:
                    chunks = fulls[:-1] + trims + [fulls[-1]]
                else:
                    # q-tile 0: single full chunk, so no room to sandwich the
                    # diagonal trim -- widen it to a full mask-1 chunk instead
                    chunks = fulls + [(kc, 1, 0, QB) for (kc, _, _, _) in trims]
                nch = len(chunks)
                # pack chunks into score/exp groups of <= SB columns so one
                # [P, <=512] exp serves several chunks (Act paces attention)
                groups = []  # per chunk: (gid, col offset); plus per-group width
                gw, gid = 0, 0
                gwidths = []
                for c in chunks:
                    if gw + c[3] > SB:
                        gwidths.append(gw)
                        gid, gw = gid + 1, 0
                    groups.append((gid, gw))
                    gw += c[3]
                gwidths.append(gw)
                glast = [i for i in range(nch)
                         if i + 1 == nch or groups[i + 1][0] != groups[i][0]]
                ao0 = pp.tile([P, QB], F32, name=f"ao0_{q0}_{h}", tag="ao", bufs=3)
                ao1 = pp.tile([P, QB], F32, name=f"ao1_{q0}_{h}", tag="ao", bufs=3)
                pacc = sp.tile([P, QB], F32, name=f"pacc_{q0}_{h}", tag="pacc", bufs=2)
                pts = [None] * nch
                scps = {}
                for i in range(nch + LOOK):
                    if i < nch:
                        kc, mi, qoff, qw = chunks[i]
                        gid, co = groups[i]
                        if gid not in scps:
                            scp = pp.tile([P, SB], F32, name=f"sc_{q0}_{h}_{gid}",
                                          tag="sc", bufs=3)
                            ptg = sp.tile([P, SB], BF16, name=f"pt_{q0}_{h}_{gid}",
                                          tag="pt", bufs=6)
                            scps[gid] = (scp, ptg)
                        scp, ptg = scps[gid]
                        qsl = ds(q0 + qoff, qw)
                        csl = ds(co, qw)
                        nc.tensor.matmul(scp[:, csl], k_sb[0][:, ts(kc, P)],
                                         q_sb[2 * h][:, qsl], start=True, stop=False)
                        nc.tensor.matmul(scp[:, csl], k_sb[1][:, ts(kc, P)],
                                         q_sb[2 * h + 1][:, qsl], start=False, stop=True)
                        pts[i] = (ptg, co)
                        if i in glast:
                            w = gwidths[gid]
                            nc.scalar.activation(ptg[:, 0:w], scp[:, 0:w], AF.Exp)
                            # masks + denominator adds for the group's chunks
                            for i2 in range(nch):
                                if groups[i2][0] != gid:
                                    continue
                                kc2, mi2, qoff2, qw2 = chunks[i2]
                                co2 = groups[i2][1]
                                live = ptg[:, ds(co2, qw2)]
                                if mi2 is not None:
                                    nc.vector.tensor_mul(
                                        live, live, msk_sb[:, ds(mi2 * SB, qw2)])
                                pql = ds(qoff2, qw2)
                                if i2 == 0:
                                    nc.vector.tensor_copy(pacc[:, pql], live)
                                else:
                                    nc.vector.tensor_add(pacc[:, pql],
                                                         pacc[:, pql], live)
                    j = i - LOOK
                    if 0 <= j < nch:
                        kc, mi, qoff, qw = chunks[j]
                        ptg, co = pts[j]
                        first, last = j == 0, j == nch - 1
                        osl, psl = ds(qoff, qw), ds(co, qw)
                        nc.tensor.matmul(ao0[:, osl], v_sb[kc][:, 0:P], ptg[:, psl],
                                         start=first, stop=last)
                        nc.tensor.matmul(ao1[:, osl], v_sb[kc][:, P:DH], ptg[:, psl],
                                         start=first, stop=last)
                dsum = sp.tile([P, QB], F32, name=f"ds_{q0}_{h}", tag="dsum", bufs=2)
                nc.gpsimd.partition_all_reduce(dsum[:], pacc[:], P,
                                               bass_isa.ReduceOp.add)
                db = sp.tile([P, QB], F32, name=f"db_{q0}_{h}", tag="db", bufs=2)
                nc.vector.reciprocal(db[:], dsum[:])
                qslice = ds(q0, QB)
                nc.vector.tensor_mul(ao_sb[2 * h][:, qslice], ao0[:], db[:])
                nc.vector.tensor_mul(ao_sb[2 * h + 1][:, qslice], ao1[:], db[:])

        # =================== main schedule ===================
        for blk in range(NSB):
            p1(blk)
            if blk == 0:
                nc.sync.dma_start(msk_sb[:].rearrange("p (m j) -> p m j", m=8),
                                  masks.rearrange("m p j -> p m j"))
                nc.sync.dma_start(wo_sb[:].rearrange("p (c e) -> p c e", c=4),
                                  woT.rearrange("(c p) e -> p c e", p=P))
            if blk > 0:
                p3(blk - 1)
            # prefetch next block's x/trig AFTER p3's out-DMAs are queued,
            # so they drain during attention when the DMA engines are idle
            if blk + 1 < NSB:
                xt = sp.tile([P, NDC * SB], BF16, name=f"xt{blk+1}", tag="xt", bufs=2)
                for g in range(2):
                    nc.sync.dma_start(
                        xt[:, ds(g * 8 * SB, 8 * SB)].rearrange("p (c s) -> p c s", c=8),
                        xT_r[:, ds(8 * g, 8), ds((blk + 1) * SB, SB)])
                xts[blk + 1] = xt
                tgt = sp.tile([P, 2 * SB], BF16, name=f"tg{blk+1}", tag="tg", bufs=2)
                nc.sync.dma_start(tgt[:].rearrange("p (r s) -> p r s", r=2),
                                  trig.rearrange("r p s -> p r s")[:, :, ds((blk + 1) * SB, SB)])
                tgs[blk + 1] = tgt
            p2(blk)
        p3(NSB - 1)

    # Pin the activation-table pass to the one set containing ln+exp+copy
    # (canonical set ids preserved; other sets hidden so the per-block
    # ln/exp sequence never reloads tables). Restored right after compile.
    import concourse.bacc as _bacc_mod

    orig_gat = _bacc_mod.get_activation_tables

    def pinned_gat(arch):
        return {name: (funcs if name == "natural_log_exp_and_others" else set())
                for name, funcs in orig_gat(arch).items()}

    _bacc_mod.get_activation_tables = pinned_gat
    try:
        nc.compile()
    finally:
        _bacc_mod.get_activation_tables = orig_gat
    return nc


_NC = None
LAST_RESULT = None


def _get_nc():
    global _NC
    if _NC is None:
        _NC = _build()
    return _NC


def _host_tables(q_norm_w, k_norm_w):
    qw, kw = np.asarray(q_norm_w, np.float64), np.asarray(k_norm_w, np.float64)
    # device shares one cos/sin table across q/k and both rotary halves;
    # requires uniform (1 + w) factors (true for Gemma-zero-init norm weights)
    assert np.allclose(qw, qw[0]) and np.allclose(kw, kw[0]) and np.allclose(qw[0], kw[0]), \
        "non-uniform q/k norm weights need the 8-row trig layout"
    c = 1.0 + qw[0]
    inv_freq = 1.0 / (ROPE_BASE ** (np.arange(0, DH, 2, dtype=np.float64) / DH))
    freqs = np.outer(np.arange(S, dtype=np.float64), inv_freq)   # [S, DH/2]
    cos = (np.cos(freqs) * c).T.astype(np.float32)               # [DH/2, S]
    sin = (np.sin(freqs) * c).T.astype(np.float32)
    trig = np.stack([cos, sin]).astype(NPBF16)                   # [2, 128, S]

    i = np.arange(P)[:, None]
    j = np.arange(SB)[None, :]
    mrows = [(j >= i + P * o) for o in range(4)] + [(j <= i + P * o - 1) for o in range(4)]
    masks = np.stack(mrows).astype(NPBF16)
    ones = np.ones((P, 1), NPBF16)
    return trig, masks, ones


def kernel(hidden_states, Wq, Wk, Wv, Wo, q_norm_w, k_norm_w):
    global LAST_RESULT
    nc = _get_nc()
    trig, masks, ones = _host_tables(np.asarray(q_norm_w), np.asarray(k_norm_w))

    xTs = [np.ascontiguousarray(np.asarray(hidden_states)[b].T).astype(NPBF16)
           for b in range(B)]
    in_maps = []
    for core in range(8):
        b, g = core // 4, core % 4
        in_maps.append({
            "xT": xTs[b],
            "wqT": np.ascontiguousarray(np.asarray(Wq)[g * EQ:(g + 1) * EQ, :].T).astype(NPBF16),
            "wkT": np.ascontiguousarray(np.asarray(Wk)[g * DH:(g + 1) * DH, :].T).astype(NPBF16),
            "wvT": np.ascontiguousarray(np.asarray(Wv)[g * DH:(g + 1) * DH, :].T).astype(NPBF16),
            "woT": np.ascontiguousarray(np.asarray(Wo)[:, g * EQ:(g + 1) * EQ].T).astype(NPBF16),
            "trig": trig,
            "masks": masks,
            "onesd": ones,
        })

    LAST_RESULT = run_bass_kernel_spmd(nc, in_maps, list(range(8)))
    res = LAST_RESULT.results
    outs = []
    for b in range(B):
        acc = np.zeros((D, S), np.float32)
        for g in range(4):
            acc += res[4 * b + g]["out"].astype(np.float32)
        outs.append(acc.T)
    return np.stack(outs).astype(np.float32)


# revision 73
# speedup vs baseline: 1.0041x; 1.0041x over previous
"""Gemma sliding-window attention layer on 8 Trainium2 NeuronCores.

Sharding: data-parallel over batch (B=2) x tensor-parallel over heads
(4 groups: 2 q heads + 1 kv head each) = 8 cores. Each core computes a
partial o-proj output [D, S] in bf16; host sums the 4 TP partials per
batch in fp32 and transposes back to [S, D].

Schedule (per 512-wide seq block b): proj(b) -> o-proj(b-1) -> attn(b),
so the rmsnorm/rope chain of block b and the softmax normalize of block
b-1 hide behind independent PE work. Within attention, score matmuls
run a fixed lookahead ahead of the PV matmuls so exp/mask latency never
stalls the PE. RMSNorm rinv = exp(-0.5*ln(ms)) keeps the activation
engine on a single function table (ln+exp share one set; sqrt doesn't).
RoPE runs in bf16 on SBUF for the DVE 2x mode. Half-width diagonal /
window-edge score tiles write into persistent half-zeroed pt tiles so
the PV accumulation stays full-width.
"""

import sys

sys.path.insert(0, "/opt/trn_rl_repo")

import math
from contextlib import ExitStack

import numpy as np
import ml_dtypes

import concourse.bass as bass
import concourse.bass_isa as bass_isa
import concourse.tile as tile
from concourse import bacc, mybir
from concourse.bass import ds, ts
from concourse.bass_utils import run_bass_kernel_spmd

BF16 = mybir.dt.bfloat16
F32 = mybir.dt.float32
NPBF16 = ml_dtypes.bfloat16
AF = mybir.ActivationFunctionType

H, KVH, DH, SW = 8, 4, 256, 1024
B, S, D = 2, 2048, 2048
EPS = 1e-6
ROPE_BASE = 10000.0
P = 128
SB = 512          # s-block width
NSB = S // SB     # 4
NDC = D // P      # 16 contraction chunks
EQ = 2 * DH       # per-core q width (2 heads)
QB = 256          # attention q-tile width
KBIAS = -math.log(16.0)  # folds the 1/sqrt(256) attn scale into k's rinv
LOOK = 3          # score-tile lookahead in the attention pipeline


def _kchunks(q0):
    """k-chunks for q-tile [q0, q0+QB).

    Returns (kc, mi, qoff, qw): k-chunk index, mask index or None, live
    q-column offset/width.  pT tile [k=128 (i), q (j)]: kpos = 128*kc + i,
    qpos = q0 + qoff + j; valid iff 0 <= qpos - kpos < SW.
    Masks: 0 -> j >= i (causal diag), 4 -> j <= i-1, 5 -> j <= i+127.
    d = 128*kc - q0:  d = 0 full mask0; d = +128 half tile (qoff 128,
    mask0 cols 0:128); d + SW = 0 half tile (qoff 0, mask4 cols 0:128);
    d + SW = 128 full tile mask5; other in-range chunks are mask-free.
    """
    res = []
    for KC in range(max(0, q0 - SW), q0 + QB, P):
        d = KC - q0
        if d == P:
            res.append((KC // P, 0, P, P))
        elif d == 0:
            res.append((KC // P, 0, 0, QB))
        elif d + SW == 0:
            res.append((KC // P, 4, 0, P))
        elif d + SW == P:
            res.append((KC // P, 5, 0, QB))
        else:
            res.append((KC // P, None, 0, QB))
    return res


def _build():
    nc = bacc.Bacc("TRN2", target_bir_lowering=False, debug=False)

    xT = nc.dram_tensor("xT", [D, S], BF16, kind="ExternalInput")
    wqT = nc.dram_tensor("wqT", [D, EQ], BF16, kind="ExternalInput")
    wkT = nc.dram_tensor("wkT", [D, DH], BF16, kind="ExternalInput")
    wvT = nc.dram_tensor("wvT", [D, DH], BF16, kind="ExternalInput")
    woT = nc.dram_tensor("woT", [EQ, D], BF16, kind="ExternalInput")
    trig = nc.dram_tensor("trig", [2, P, S], BF16, kind="ExternalInput")  # cos, sin
    masks = nc.dram_tensor("masks", [8, P, SB], BF16, kind="ExternalInput")
    onesd = nc.dram_tensor("onesd", [P, 1], BF16, kind="ExternalInput")
    out = nc.dram_tensor("out", [D, S], BF16, kind="ExternalOutput")

    # const tiles for activation bias operands
    for val in (EPS, KBIAS):
        t = nc.alloc_sbuf_tensor(f"const-bias-{val}", [P, 1], F32)
        nc.gpsimd.memset(t.ap(), val)
        nc.const_aps.aps[(F32, val)] = t.ap()
    nc.all_engine_barrier()

    with tile.TileContext(nc) as tc, ExitStack() as ctx:
        sp = ctx.enter_context(tc.tile_pool(name="sp", bufs=2))    # SBUF
        pp = ctx.enter_context(tc.tile_pool(name="pp", bufs=2, space="PSUM"))

        # ---- persistent SBUF tiles ----
        wq_sb = sp.tile([P, NDC * EQ], BF16, name="wq", tag="wq", bufs=1)
        wk_sb = sp.tile([P, NDC * DH], BF16, name="wk", tag="wk", bufs=1)
        wv_sb = sp.tile([P, NDC * DH], BF16, name="wv", tag="wv", bufs=1)
        wo_sb = sp.tile([P, 4 * D], BF16, name="wo", tag="wo", bufs=1)
        msk_sb = sp.tile([P, 8 * SB], BF16, name="msk", tag="msk", bufs=1)
        q_sb = [sp.tile([P, S], BF16, name=f"qsb{i}", tag="qsb", bufs=4) for i in range(4)]
        k_sb = [sp.tile([P, S], BF16, name=f"ksb{i}", tag="ksb", bufs=2) for i in range(2)]
        v_sb = [sp.tile([P, DH], BF16, name=f"vsb{i}", tag="vsb", bufs=NDC) for i in range(NDC)]
        ao_sb = [sp.tile([P, S], BF16, name=f"aosb{i}", tag="aosb", bufs=4) for i in range(4)]


        # ---- startup loads: k-ent weights + x block 0 first, fine-grained ----
        xT_r = xT.rearrange("(c p) s -> p c s", p=P)
        wk_r = wkT.rearrange("(c p) e -> p c e", p=P)
        wv_r = wvT.rearrange("(c p) e -> p c e", p=P)
        wq_r = wqT.rearrange("(c p) e -> p c e", p=P)

        xt0 = sp.tile([P, NDC * SB], BF16, name="xt0", tag="xt", bufs=2)
        nc.sync.dma_start(
            wk_sb[:, ds(0, 4 * DH)].rearrange("p (c e) -> p c e", c=4),
            wk_r[:, ds(0, 4), :])
        nc.sync.dma_start(
            xt0[:, ds(0, SB)].rearrange("p (c s) -> p c s", c=1),
            xT_r[:, ds(0, 1), ds(0, SB)])
        nc.sync.dma_start(
            xt0[:, ds(SB, SB)].rearrange("p (c s) -> p c s", c=1),
            xT_r[:, ds(1, 1), ds(0, SB)])
        for g in range(1, 8):
            nc.sync.dma_start(
                xt0[:, ds(g * 2 * SB, 2 * SB)].rearrange("p (c s) -> p c s", c=2),
                xT_r[:, ds(2 * g, 2), ds(0, SB)])
            if g in (1, 3, 5):
                gk = (g + 1) // 2
                nc.sync.dma_start(
                    wk_sb[:, ds(gk * 4 * DH, 4 * DH)].rearrange("p (c e) -> p c e", c=4),
                    wk_r[:, ds(4 * gk, 4), :])
        for g in range(4):
            nc.sync.dma_start(
                wq_sb[:, ds(g * 4 * EQ, 4 * EQ)].rearrange("p (c e) -> p c e", c=4),
                wq_r[:, ds(4 * g, 4), :])
        for g in range(2):
            nc.sync.dma_start(
                wv_sb[:, ds(g * 8 * DH, 8 * DH)].rearrange("p (c e) -> p c e", c=8),
                wv_r[:, ds(8 * g, 8), :])
        tg0 = sp.tile([P, 2 * SB], BF16, name="tg0", tag="tg", bufs=2)
        nc.sync.dma_start(tg0[:].rearrange("p (r s) -> p r s", r=2),
                          trig.rearrange("r p s -> p r s")[:, :, ds(0, SB)])

        def wq_ap(dc, eoff):
            return wq_sb[:, ds(dc * EQ + eoff, P)]

        def wk_ap(dc, eoff):
            return wk_sb[:, ds(dc * DH + eoff, P)]

        out_r = out.rearrange("(g p) s -> p g s", p=P)

        xts = {0: xt0}
        tgs = {0: tg0}

        # =================== phase emitters ===================

        def p1(blk):
            """Projections + rmsnorm + rope for seq block blk (k, V, q0, q1)."""
            xt = xts.pop(blk)
            tgt = tgs.pop(blk)
            sblice = ds(blk * SB, SB)
            tcos, tsin = tgt[:, 0:SB], tgt[:, SB : 2 * SB]

            ents = [("k", wk_ap, 0, 0), ("q0", wq_ap, 0, 32), ("q1", wq_ap, DH, 64)]

            def proj(ent, w_ap, eoff):
                pa = pp.tile([P, SB], F32, name=f"pa_{blk}_{ent}", tag="pj", bufs=3)
                pb = pp.tile([P, SB], F32, name=f"pb_{blk}_{ent}", tag="pj", bufs=3)
                for dc in range(NDC):
                    nc.tensor.matmul(pa[:], w_ap(dc, eoff), xt[:, ds(dc * SB, SB)],
                                     start=(dc == 0), stop=(dc == NDC - 1))
                pab = sp.tile([P, SB], BF16, name=f"pab_{blk}_{ent}", tag="pr", bufs=6)
                nc.vector.tensor_copy(pab[:], pa[:])
                for dc in range(NDC):
                    nc.tensor.matmul(pb[:], w_ap(dc, eoff + P), xt[:, ds(dc * SB, SB)],
                                     start=(dc == 0), stop=(dc == NDC - 1))
                pbb = sp.tile([P, SB], BF16, name=f"pbb_{blk}_{ent}", tag="pr", bufs=6)
                nc.vector.tensor_copy(pbb[:], pb[:])
                sqa = sp.tile([P, SB], BF16, name=f"sqa_{blk}_{ent}", tag="sq", bufs=4)
                sqb = sp.tile([P, SB], BF16, name=f"sqb_{blk}_{ent}", tag="sq", bufs=4)
                nc.vector.tensor_mul(sqa[:], pab[:], pab[:])
                nc.vector.tensor_mul(sqb[:], pbb[:], pbb[:])
                return pab, pbb, sqa, sqb

            def emit_sums(ent, sqa, sqb):
                # partition sums on the (otherwise idle) gpsimd engine; the
                # all-reduce result arrives already broadcast over partitions,
                # so ln/exp produce the full [P, SB] rinv with no bcast step.
                ra = sp.tile([P, SB], F32, name=f"ra_{blk}_{ent}", tag="ra", bufs=2)
                rb = sp.tile([P, SB], F32, name=f"rb_{blk}_{ent}", tag="ra", bufs=2)
                nc.gpsimd.partition_all_reduce(ra[:], sqa[:], P, bass_isa.ReduceOp.add)
                nc.gpsimd.partition_all_reduce(rb[:], sqb[:], P, bass_isa.ReduceOp.add)
                rs = sp.tile([P, SB], F32, name=f"rs_{blk}_{ent}", tag="rs", bufs=3)
                nc.gpsimd.tensor_add(rs[:], ra[:], rb[:])
                return rs

            def emit_ln(ent, rs):
                tln = sp.tile([P, SB], F32, name=f"tln_{blk}_{ent}", tag="tl", bufs=3)
                nc.scalar.activation(tln[:], rs[:], AF.Ln, bias=EPS, scale=1.0 / DH)
                return tln

            def emit_exp(ent, tln):
                qb = sp.tile([P, SB], BF16, name=f"qb_{blk}_{ent}", tag="qb", bufs=4)
                nc.scalar.activation(qb[:], tln[:], AF.Exp, scale=-0.5,
                                     bias=(KBIAS if ent == "k" else 0.0))
                return qb

            def emit_ropemuls(ent, qb, pab, pbb, lo, hi):
                ta = sp.tile([P, SB], BF16, name=f"ta_{blk}_{ent}", tag="rt", bufs=8)
                tb = sp.tile([P, SB], BF16, name=f"tb_{blk}_{ent}", tag="rt", bufs=8)
                tc_ = sp.tile([P, SB], BF16, name=f"tc_{blk}_{ent}", tag="rt", bufs=8)
                nc.vector.tensor_mul(ta[:], pab[:], tcos)
                nc.vector.tensor_mul(tb[:], pbb[:], tsin)
                nc.vector.tensor_sub(tc_[:], ta[:], tb[:])
                nc.vector.tensor_mul(lo[:, sblice], tc_[:], qb[:])
                td = sp.tile([P, SB], BF16, name=f"td_{blk}_{ent}", tag="rt", bufs=8)
                te = sp.tile([P, SB], BF16, name=f"te_{blk}_{ent}", tag="rt", bufs=8)
                tf = sp.tile([P, SB], BF16, name=f"tf_{blk}_{ent}", tag="rt", bufs=8)
                nc.vector.tensor_mul(td[:], pbb[:], tcos)
                nc.vector.tensor_mul(te[:], pab[:], tsin)
                nc.vector.tensor_add(tf[:], td[:], te[:])
                nc.vector.tensor_mul(hi[:, sblice], tf[:], qb[:])

            dests = {"k": (k_sb[0], k_sb[1]), "q0": (q_sb[0], q_sb[1]),
                     "q1": (q_sb[2], q_sb[3])}
            info = {}

            def chain_act(e):
                return emit_exp(e, emit_ln(e, emit_sums(e, info[e][2], info[e][3])))

            # --- k, q0, q1 projections, then V. Each ent's sums/ln/exp chain
            # goes out as soon as its squares exist (no act-table switches:
            # the table set is pinned, see compile below). Rope muls (DVE) are
            # emitted after the V copies so the pv/op psum rings never wait
            # behind them in the DVE queue; block 0 (no o-proj cover ahead of
            # attention) emits k/q0 ropes early instead.
            qbs = {}
            info["k"] = proj("k", wk_ap, 0)
            info["q0"] = proj("q0", wq_ap, 0)
            qbs["k"] = chain_act("k")
            if blk == 0:
                emit_ropemuls("k", qbs["k"], info["k"][0], info["k"][1], *dests["k"])
            info["q1"] = proj("q1", wq_ap, DH)
            qbs["q0"] = chain_act("q0")
            if blk == 0:
                emit_ropemuls("q0", qbs["q0"], info["q0"][0], info["q0"][1],
                              *dests["q0"])

            for sc in range(SB // P):
                pv = pp.tile([P, DH], F32, name=f"pv_{blk}_{sc}", tag="sc", bufs=2)
                for dc in range(NDC):
                    nc.tensor.matmul(pv[:], xt[:, ds(dc * SB + sc * P, P)],
                                     wv_sb[:, ds(dc * DH, DH)],
                                     start=(dc == 0), stop=(dc == NDC - 1))
                if sc == 0:
                    qbs["q1"] = chain_act("q1")
                nc.vector.tensor_copy(v_sb[blk * (SB // P) + sc][:], pv[:])
            late = ("q1",) if blk == 0 else ("k", "q0", "q1")
            for e in late:
                emit_ropemuls(e, qbs[e], info[e][0], info[e][1], *dests[e])

        def p3(blk):
            """o-projection for block blk -> DRAM. All copies on Act: its
            queue is clear here (norm batch finished during V), while DVE
            holds the rope batch. DMA per 4-op group keeps HWDGE slots few."""
            sblice = ds(blk * SB, SB)
            finegrain = blk == NSB - 1  # last block: per-op DMA, short drain
            for g4 in range(4):
                ob4 = sp.tile([P, 4 * SB], BF16, name=f"ob_{blk}_{g4}", tag="ob", bufs=2)
                for j in range(4):
                    dmc = 4 * g4 + j
                    op = pp.tile([P, SB], F32, name=f"op_{blk}_{dmc}", tag="pj", bufs=3)
                    for ec in range(4):
                        nc.tensor.matmul(op[:], wo_sb[:, ds(ec * D + dmc * P, P)],
                                         ao_sb[ec][:, sblice],
                                         start=(ec == 0), stop=(ec == 3))
                    if finegrain and g4 == 3 and j == 3:
                        nc.vector.tensor_copy(ob4[:, ds(j * SB, SB)], op[:])
                    else:
                        nc.scalar.copy(ob4[:, ds(j * SB, SB)], op[:])
                    if finegrain:
                        nc.sync.dma_start(
                            out_r[:, ds(dmc, 1), sblice],
                            ob4[:, ds(j * SB, SB)].rearrange("p (g s) -> p g s", g=1))
                if not finegrain:
                    nc.sync.dma_start(
                        out_r[:, ds(4 * g4, 4), sblice],
                        ob4[:].rearrange("p (g s) -> p g s", g=4))

        def p2(blk):
            """Attention for the two q-tiles of block blk.

            Chunk emission order sandwiches the half-width (trimmed) chunks
            between full ones, so the PV psum accumulation starts on the
            first full chunk (full width) and stops on the last full chunk,
            with the trims accumulating into column sub-ranges in between.
            q0=0 has a single full chunk, so its diagonal trim keeps the
            legacy zero-half pth tile and accumulates at full width.
            """
            for it, (sub, h) in enumerate([(s_, h_) for h_ in range(2) for s_ in range(2)]):
                q0 = blk * SB + sub * QB
                raw = _kchunks(q0)
                fulls = [c for c in raw if c[3] == QB]
                trims = [c for c in raw if c[3] != QB]
                if len(fulls) >= 2:
                    chunks = fulls[:-1] + trims + [fulls[-1]]
                else:
                    # q-tile 0: single full chunk, so no room to sandwich the
                    # diagonal trim -- widen it to a full mask-1 chunk instead
                    chunks = fulls + [(kc, 1, 0, QB) for (kc, _, _, _) in trims]
                nch = len(chunks)
                # pack chunks into score/exp groups of <= SB columns so one
                # [P, <=512] exp serves several chunks (Act paces attention)
                groups = []  # per chunk: (gid, col offset); plus per-group width
                gw, gid = 0, 0
                gwidths = []
                for c in chunks:
                    if gw + c[3] > SB:
                        gwidths.append(gw)
                        gid, gw = gid + 1, 0
                    groups.append((gid, gw))
                    gw += c[3]
                gwidths.append(gw)
                glast = [i for i in range(nch)
                         if i + 1 == nch or groups[i + 1][0] != groups[i][0]]
                ao0 = pp.tile([P, QB], F32, name=f"ao0_{q0}_{h}", tag="ao", bufs=3)
                ao1 = pp.tile([P, QB], F32, name=f"ao1_{q0}_{h}", tag="ao", bufs=3)
                pacc = sp.tile([P, QB], F32, name=f"pacc_{q0}_{h}", tag="pacc", bufs=2)
                pts = [None] * nch
                scps = {}
                for i in range(nch + LOOK):
                    if i < nch:
                        kc, mi, qoff, qw = chunks[i]
                        gid, co = groups[i]
                        if gid not in scps:
                            scp = pp.tile([P, SB], F32, name=f"sc_{q0}_{h}_{gid}",
                                          tag="sc", bufs=2)
                            ptg = sp.tile([P, SB], BF16, name=f"pt_{q0}_{h}_{gid}",
                                          tag="pt", bufs=6)
                            scps[gid] = (scp, ptg)
                        scp, ptg = scps[gid]
                        qsl = ds(q0 + qoff, qw)
                        csl = ds(co, qw)
                        nc.tensor.matmul(scp[:, csl], k_sb[0][:, ts(kc, P)],
                                         q_sb[2 * h][:, qsl], start=True, stop=False)
                        nc.tensor.matmul(scp[:, csl], k_sb[1][:, ts(kc, P)],
                                         q_sb[2 * h + 1][:, qsl], start=False, stop=True)
                        pts[i] = (ptg, co)
                        if i in glast:
                            w = gwidths[gid]
                            nc.scalar.activation(ptg[:, 0:w], scp[:, 0:w], AF.Exp)
                            # masks + denominator adds for the group's chunks
                            for i2 in range(nch):
                                if groups[i2][0] != gid:
                                    continue
                                kc2, mi2, qoff2, qw2 = chunks[i2]
                                co2 = groups[i2][1]
                                live = ptg[:, ds(co2, qw2)]
                                if mi2 is not None:
                                    nc.vector.tensor_mul(
                                        live, live, msk_sb[:, ds(mi2 * SB, qw2)])
                                pql = ds(qoff2, qw2)
                                if i2 == 0:
                                    nc.vector.tensor_copy(pacc[:, pql], live)
                                else:
                                    nc.vector.tensor_add(pacc[:, pql],
                                                         pacc[:, pql], live)
                    j = i - LOOK
                    if 0 <= j < nch:
                        kc, mi, qoff, qw = chunks[j]
                        ptg, co = pts[j]
                        first, last = j == 0, j == nch - 1
                        osl, psl = ds(qoff, qw), ds(co, qw)
                        nc.tensor.matmul(ao0[:, osl], v_sb[kc][:, 0:P], ptg[:, psl],
                                         start=first, stop=last)
                        nc.tensor.matmul(ao1[:, osl], v_sb[kc][:, P:DH], ptg[:, psl],
                                         start=first, stop=last)
                dsum = sp.tile([P, QB], F32, name=f"ds_{q0}_{h}", tag="dsum", bufs=2)
                nc.gpsimd.partition_all_reduce(dsum[:], pacc[:], P,
                                               bass_isa.ReduceOp.add)
                db = sp.tile([P, QB], F32, name=f"db_{q0}_{h}", tag="db", bufs=2)
                nc.vector.reciprocal(db[:], dsum[:])
                qslice = ds(q0, QB)
                nc.vector.tensor_mul(ao_sb[2 * h][:, qslice], ao0[:], db[:])
                nc.vector.tensor_mul(ao_sb[2 * h + 1][:, qslice], ao1[:], db[:])

        # =================== main schedule ===================
        for blk in range(NSB):
            p1(blk)
            if blk == 0:
                nc.sync.dma_start(msk_sb[:].rearrange("p (m j) -> p m j", m=8),
                                  masks.rearrange("m p j -> p m j"))
                nc.sync.dma_start(wo_sb[:].rearrange("p (c e) -> p c e", c=4),
                                  woT.rearrange("(c p) e -> p c e", p=P))
            if blk > 0:
                p3(blk - 1)
            # prefetch next block's x/trig AFTER p3's out-DMAs are queued,
            # so they drain during attention when the DMA engines are idle
            if blk + 1 < NSB:
                xt = sp.tile([P, NDC * SB], BF16, name=f"xt{blk+1}", tag="xt", bufs=2)
                for g in range(2):
                    nc.sync.dma_start(
                        xt[:, ds(g * 8 * SB, 8 * SB)].rearrange("p (c s) -> p c s", c=8),
                        xT_r[:, ds(8 * g, 8), ds((blk + 1) * SB, SB)])
                xts[blk + 1] = xt
                tgt = sp.tile([P, 2 * SB], BF16, name=f"tg{blk+1}", tag="tg", bufs=2)
                nc.sync.dma_start(tgt[:].rearrange("p (r s) -> p r s", r=2),
                                  trig.rearrange("r p s -> p r s")[:, :, ds((blk + 1) * SB, SB)])
                tgs[blk + 1] = tgt
            p2(blk)
        p3(NSB - 1)

    # Pin the activation-table pass to the one set containing ln+exp+copy
    # (canonical set ids preserved; other sets hidden so the per-block
    # ln/exp sequence never reloads tables). Restored right after compile.
    import concourse.bacc as _bacc_mod

    orig_gat = _bacc_mod.get_activation_tables

    def pinned_gat(arch):
        return {name: (funcs if name == "natural_log_exp_and_others" else set())
                for name, funcs in orig_gat(arch).items()}

    _bacc_mod.get_activation_tables = pinned_gat
    try:
        nc.compile()
    finally:
        _bacc_mod.get_activation_tables = orig_gat
    return nc


_NC = None
LAST_RESULT = None


def _get_nc():
    global _NC
    if _NC is None:
        _NC = _build()
    return _NC


def _host_tables(q_norm_w, k_norm_w):
    qw, kw = np.asarray(q_norm_w, np.float64), np.asarray(k_norm_w, np.float64)
    # device shares one cos/sin table across q/k and both rotary halves;
    # requires uniform (1 + w) factors (true for Gemma-zero-init norm weights)
    assert np.allclose(qw, qw[0]) and np.allclose(kw, kw[0]) and np.allclose(qw[0], kw[0]), \
        "non-uniform q/k norm weights need the 8-row trig layout"
    c = 1.0 + qw[0]
    inv_freq = 1.0 / (ROPE_BASE ** (np.arange(0, DH, 2, dtype=np.float64) / DH))
    freqs = np.outer(np.arange(S, dtype=np.float64), inv_freq)   # [S, DH/2]
    cos = (np.cos(freqs) * c).T.astype(np.float32)               # [DH/2, S]
    sin = (np.sin(freqs) * c).T.astype(np.float32)
    trig = np.stack([cos, sin]).astype(NPBF16)                   # [2, 128, S]

    i = np.arange(P)[:, None]
    j = np.arange(SB)[None, :]
    mrows = [(j >= i + P * o) for o in range(4)] + [(j <= i + P * o - 1) for o in range(4)]
    masks = np.stack(mrows).astype(NPBF16)
    ones = np.ones((P, 1), NPBF16)
    return trig, masks, ones


def kernel(hidden_states, Wq, Wk, Wv, Wo, q_norm_w, k_norm_w):
    global LAST_RESULT
    nc = _get_nc()
    trig, masks, ones = _host_tables(np.asarray(q_norm_w), np.asarray(k_norm_w))

    xTs = [np.ascontiguousarray(np.asarray(hidden_states)[b].T).astype(NPBF16)
           for b in range(B)]
    in_maps = []
    for core in range(8):
        b, g = core // 4, core % 4
        in_maps.append({
            "xT": xTs[b],
            "wqT": np.ascontiguousarray(np.asarray(Wq)[g * EQ:(g + 1) * EQ, :].T).astype(NPBF16),
            "wkT": np.ascontiguousarray(np.asarray(Wk)[g * DH:(g + 1) * DH, :].T).astype(NPBF16),
            "wvT": np.ascontiguousarray(np.asarray(Wv)[g * DH:(g + 1) * DH, :].T).astype(NPBF16),
            "woT": np.ascontiguousarray(np.asarray(Wo)[:, g * EQ:(g + 1) * EQ].T).astype(NPBF16),
            "trig": trig,
            "masks": masks,
            "onesd": ones,
        })

    LAST_RESULT = run_bass_kernel_spmd(nc, in_maps, list(range(8)))
    res = LAST_RESULT.results
    outs = []
    for b in range(B):
        acc = np.zeros((D, S), np.float32)
        for g in range(4):
            acc += res[4 * b + g]["out"].astype(np.float32)
        outs.append(acc.T)
    return np.stack(outs).astype(np.float32)


# revision 74
# speedup vs baseline: 1.0062x; 1.0021x over previous
"""Gemma sliding-window attention layer on 8 Trainium2 NeuronCores.

Sharding: data-parallel over batch (B=2) x tensor-parallel over heads
(4 groups: 2 q heads + 1 kv head each) = 8 cores. Each core computes a
partial o-proj output [D, S] in bf16; host sums the 4 TP partials per
batch in fp32 and transposes back to [S, D].

Schedule (per 512-wide seq block b): proj(b) -> o-proj(b-1) -> attn(b),
so the rmsnorm/rope chain of block b and the softmax normalize of block
b-1 hide behind independent PE work. Within attention, score matmuls
run a fixed lookahead ahead of the PV matmuls so exp/mask latency never
stalls the PE. RMSNorm rinv = exp(-0.5*ln(ms)) keeps the activation
engine on a single function table (ln+exp share one set; sqrt doesn't).
RoPE runs in bf16 on SBUF for the DVE 2x mode. Half-width diagonal /
window-edge score tiles write into persistent half-zeroed pt tiles so
the PV accumulation stays full-width.
"""

import sys

sys.path.insert(0, "/opt/trn_rl_repo")

import math
from contextlib import ExitStack

import numpy as np
import ml_dtypes

import concourse.bass as bass
import concourse.bass_isa as bass_isa
import concourse.tile as tile
from concourse import bacc, mybir
from concourse.bass import ds, ts
from concourse.bass_utils import run_bass_kernel_spmd

BF16 = mybir.dt.bfloat16
F32 = mybir.dt.float32
NPBF16 = ml_dtypes.bfloat16
AF = mybir.ActivationFunctionType

H, KVH, DH, SW = 8, 4, 256, 1024
B, S, D = 2, 2048, 2048
EPS = 1e-6
ROPE_BASE = 10000.0
P = 128
SB = 512          # s-block width
NSB = S // SB     # 4
NDC = D // P      # 16 contraction chunks
EQ = 2 * DH       # per-core q width (2 heads)
QB = 256          # attention q-tile width
KBIAS = -math.log(16.0)  # folds the 1/sqrt(256) attn scale into k's rinv
LOOK = 3          # score-tile lookahead in the attention pipeline


def _kchunks(q0):
    """k-chunks for q-tile [q0, q0+QB).

    Returns (kc, mi, qoff, qw): k-chunk index, mask index or None, live
    q-column offset/width.  pT tile [k=128 (i), q (j)]: kpos = 128*kc + i,
    qpos = q0 + qoff + j; valid iff 0 <= qpos - kpos < SW.
    Masks: 0 -> j >= i (causal diag), 4 -> j <= i-1, 5 -> j <= i+127.
    d = 128*kc - q0:  d = 0 full mask0; d = +128 half tile (qoff 128,
    mask0 cols 0:128); d + SW = 0 half tile (qoff 0, mask4 cols 0:128);
    d + SW = 128 full tile mask5; other in-range chunks are mask-free.
    """
    res = []
    for KC in range(max(0, q0 - SW), q0 + QB, P):
        d = KC - q0
        if d == P:
            res.append((KC // P, 0, P, P))
        elif d == 0:
            res.append((KC // P, 0, 0, QB))
        elif d + SW == 0:
            res.append((KC // P, 4, 0, P))
        elif d + SW == P:
            res.append((KC // P, 5, 0, QB))
        else:
            res.append((KC // P, None, 0, QB))
    return res


def _build():
    nc = bacc.Bacc("TRN2", target_bir_lowering=False, debug=False)

    xT = nc.dram_tensor("xT", [D, S], BF16, kind="ExternalInput")
    wqT = nc.dram_tensor("wqT", [D, EQ], BF16, kind="ExternalInput")
    wkT = nc.dram_tensor("wkT", [D, DH], BF16, kind="ExternalInput")
    wvT = nc.dram_tensor("wvT", [D, DH], BF16, kind="ExternalInput")
    woT = nc.dram_tensor("woT", [EQ, D], BF16, kind="ExternalInput")
    trig = nc.dram_tensor("trig", [2, P, S], BF16, kind="ExternalInput")  # cos, sin
    masks = nc.dram_tensor("masks", [8, P, SB], BF16, kind="ExternalInput")
    onesd = nc.dram_tensor("onesd", [P, 1], BF16, kind="ExternalInput")
    out = nc.dram_tensor("out", [D, S], BF16, kind="ExternalOutput")

    # const tiles for activation bias operands
    for val in (EPS, KBIAS):
        t = nc.alloc_sbuf_tensor(f"const-bias-{val}", [P, 1], F32)
        nc.gpsimd.memset(t.ap(), val)
        nc.const_aps.aps[(F32, val)] = t.ap()
    nc.all_engine_barrier()

    with tile.TileContext(nc) as tc, ExitStack() as ctx:
        sp = ctx.enter_context(tc.tile_pool(name="sp", bufs=2))    # SBUF
        pp = ctx.enter_context(tc.tile_pool(name="pp", bufs=2, space="PSUM"))

        # ---- persistent SBUF tiles ----
        wq_sb = sp.tile([P, NDC * EQ], BF16, name="wq", tag="wq", bufs=1)
        wk_sb = sp.tile([P, NDC * DH], BF16, name="wk", tag="wk", bufs=1)
        wv_sb = sp.tile([P, NDC * DH], BF16, name="wv", tag="wv", bufs=1)
        wo_sb = sp.tile([P, 4 * D], BF16, name="wo", tag="wo", bufs=1)
        msk_sb = sp.tile([P, 8 * SB], BF16, name="msk", tag="msk", bufs=1)
        q_sb = [sp.tile([P, S], BF16, name=f"qsb{i}", tag="qsb", bufs=4) for i in range(4)]
        k_sb = [sp.tile([P, S], BF16, name=f"ksb{i}", tag="ksb", bufs=2) for i in range(2)]
        v_sb = [sp.tile([P, DH], BF16, name=f"vsb{i}", tag="vsb", bufs=NDC) for i in range(NDC)]
        ao_sb = [sp.tile([P, S], BF16, name=f"aosb{i}", tag="aosb", bufs=4) for i in range(4)]


        # ---- startup loads: k-ent weights + x block 0 first, fine-grained ----
        xT_r = xT.rearrange("(c p) s -> p c s", p=P)
        wk_r = wkT.rearrange("(c p) e -> p c e", p=P)
        wv_r = wvT.rearrange("(c p) e -> p c e", p=P)
        wq_r = wqT.rearrange("(c p) e -> p c e", p=P)

        xt0 = sp.tile([P, NDC * SB], BF16, name="xt0", tag="xt", bufs=2)
        nc.sync.dma_start(
            wk_sb[:, ds(0, 4 * DH)].rearrange("p (c e) -> p c e", c=4),
            wk_r[:, ds(0, 4), :])
        nc.sync.dma_start(
            xt0[:, ds(0, SB)].rearrange("p (c s) -> p c s", c=1),
            xT_r[:, ds(0, 1), ds(0, SB)])
        nc.sync.dma_start(
            xt0[:, ds(SB, SB)].rearrange("p (c s) -> p c s", c=1),
            xT_r[:, ds(1, 1), ds(0, SB)])
        for g in range(1, 8):
            nc.sync.dma_start(
                xt0[:, ds(g * 2 * SB, 2 * SB)].rearrange("p (c s) -> p c s", c=2),
                xT_r[:, ds(2 * g, 2), ds(0, SB)])
            if g in (1, 3, 5):
                gk = (g + 1) // 2
                nc.sync.dma_start(
                    wk_sb[:, ds(gk * 4 * DH, 4 * DH)].rearrange("p (c e) -> p c e", c=4),
                    wk_r[:, ds(4 * gk, 4), :])
        for g in range(4):
            nc.sync.dma_start(
                wq_sb[:, ds(g * 4 * EQ, 4 * EQ)].rearrange("p (c e) -> p c e", c=4),
                wq_r[:, ds(4 * g, 4), :])
        for g in range(2):
            nc.sync.dma_start(
                wv_sb[:, ds(g * 8 * DH, 8 * DH)].rearrange("p (c e) -> p c e", c=8),
                wv_r[:, ds(8 * g, 8), :])
        tg0 = sp.tile([P, 2 * SB], BF16, name="tg0", tag="tg", bufs=2)
        nc.sync.dma_start(tg0[:].rearrange("p (r s) -> p r s", r=2),
                          trig.rearrange("r p s -> p r s")[:, :, ds(0, SB)])

        def wq_ap(dc, eoff):
            return wq_sb[:, ds(dc * EQ + eoff, P)]

        def wk_ap(dc, eoff):
            return wk_sb[:, ds(dc * DH + eoff, P)]

        out_r = out.rearrange("(g p) s -> p g s", p=P)

        xts = {0: xt0}
        tgs = {0: tg0}

        # =================== phase emitters ===================

        def p1(blk):
            """Projections + rmsnorm + rope for seq block blk (k, V, q0, q1)."""
            xt = xts.pop(blk)
            tgt = tgs.pop(blk)
            sblice = ds(blk * SB, SB)
            tcos, tsin = tgt[:, 0:SB], tgt[:, SB : 2 * SB]

            ents = [("k", wk_ap, 0, 0), ("q0", wq_ap, 0, 32), ("q1", wq_ap, DH, 64)]

            def proj(ent, w_ap, eoff):
                pa = pp.tile([P, SB], F32, name=f"pa_{blk}_{ent}", tag="pj", bufs=4)
                pb = pp.tile([P, SB], F32, name=f"pb_{blk}_{ent}", tag="pj", bufs=4)
                for dc in range(NDC):
                    nc.tensor.matmul(pa[:], w_ap(dc, eoff), xt[:, ds(dc * SB, SB)],
                                     start=(dc == 0), stop=(dc == NDC - 1))
                pab = sp.tile([P, SB], BF16, name=f"pab_{blk}_{ent}", tag="pr", bufs=6)
                nc.vector.tensor_copy(pab[:], pa[:])
                for dc in range(NDC):
                    nc.tensor.matmul(pb[:], w_ap(dc, eoff + P), xt[:, ds(dc * SB, SB)],
                                     start=(dc == 0), stop=(dc == NDC - 1))
                pbb = sp.tile([P, SB], BF16, name=f"pbb_{blk}_{ent}", tag="pr", bufs=6)
                nc.vector.tensor_copy(pbb[:], pb[:])
                sqa = sp.tile([P, SB], BF16, name=f"sqa_{blk}_{ent}", tag="sq", bufs=4)
                sqb = sp.tile([P, SB], BF16, name=f"sqb_{blk}_{ent}", tag="sq", bufs=4)
                nc.vector.tensor_mul(sqa[:], pab[:], pab[:])
                nc.vector.tensor_mul(sqb[:], pbb[:], pbb[:])
                return pab, pbb, sqa, sqb

            def emit_sums(ent, sqa, sqb):
                # partition sums on the (otherwise idle) gpsimd engine; the
                # all-reduce result arrives already broadcast over partitions,
                # so ln/exp produce the full [P, SB] rinv with no bcast step.
                ra = sp.tile([P, SB], F32, name=f"ra_{blk}_{ent}", tag="ra", bufs=2)
                rb = sp.tile([P, SB], F32, name=f"rb_{blk}_{ent}", tag="ra", bufs=2)
                nc.gpsimd.partition_all_reduce(ra[:], sqa[:], P, bass_isa.ReduceOp.add)
                nc.gpsimd.partition_all_reduce(rb[:], sqb[:], P, bass_isa.ReduceOp.add)
                rs = sp.tile([P, SB], F32, name=f"rs_{blk}_{ent}", tag="rs", bufs=3)
                nc.gpsimd.tensor_add(rs[:], ra[:], rb[:])
                return rs

            def emit_ln(ent, rs):
                tln = sp.tile([P, SB], F32, name=f"tln_{blk}_{ent}", tag="tl", bufs=3)
                nc.scalar.activation(tln[:], rs[:], AF.Ln, bias=EPS, scale=1.0 / DH)
                return tln

            def emit_exp(ent, tln):
                qb = sp.tile([P, SB], BF16, name=f"qb_{blk}_{ent}", tag="qb", bufs=4)
                nc.scalar.activation(qb[:], tln[:], AF.Exp, scale=-0.5,
                                     bias=(KBIAS if ent == "k" else 0.0))
                return qb

            def emit_ropemuls(ent, qb, pab, pbb, lo, hi):
                ta = sp.tile([P, SB], BF16, name=f"ta_{blk}_{ent}", tag="rt", bufs=8)
                tb = sp.tile([P, SB], BF16, name=f"tb_{blk}_{ent}", tag="rt", bufs=8)
                tc_ = sp.tile([P, SB], BF16, name=f"tc_{blk}_{ent}", tag="rt", bufs=8)
                nc.vector.tensor_mul(ta[:], pab[:], tcos)
                nc.vector.tensor_mul(tb[:], pbb[:], tsin)
                nc.vector.tensor_sub(tc_[:], ta[:], tb[:])
                nc.vector.tensor_mul(lo[:, sblice], tc_[:], qb[:])
                td = sp.tile([P, SB], BF16, name=f"td_{blk}_{ent}", tag="rt", bufs=8)
                te = sp.tile([P, SB], BF16, name=f"te_{blk}_{ent}", tag="rt", bufs=8)
                tf = sp.tile([P, SB], BF16, name=f"tf_{blk}_{ent}", tag="rt", bufs=8)
                nc.vector.tensor_mul(td[:], pbb[:], tcos)
                nc.vector.tensor_mul(te[:], pab[:], tsin)
                nc.vector.tensor_add(tf[:], td[:], te[:])
                nc.vector.tensor_mul(hi[:, sblice], tf[:], qb[:])

            dests = {"k": (k_sb[0], k_sb[1]), "q0": (q_sb[0], q_sb[1]),
                     "q1": (q_sb[2], q_sb[3])}
            info = {}

            def chain_act(e):
                return emit_exp(e, emit_ln(e, emit_sums(e, info[e][2], info[e][3])))

            # --- k, q0, q1 projections, then V. Each ent's sums/ln/exp chain
            # goes out as soon as its squares exist (no act-table switches:
            # the table set is pinned, see compile below). Rope muls (DVE) are
            # emitted after the V copies so the pv/op psum rings never wait
            # behind them in the DVE queue; block 0 (no o-proj cover ahead of
            # attention) emits k/q0 ropes early instead.
            qbs = {}
            info["k"] = proj("k", wk_ap, 0)
            info["q0"] = proj("q0", wq_ap, 0)
            qbs["k"] = chain_act("k")
            if blk == 0:
                emit_ropemuls("k", qbs["k"], info["k"][0], info["k"][1], *dests["k"])
            info["q1"] = proj("q1", wq_ap, DH)
            qbs["q0"] = chain_act("q0")
            if blk == 0:
                emit_ropemuls("q0", qbs["q0"], info["q0"][0], info["q0"][1],
                              *dests["q0"])

            for sc in range(SB // P):
                pv = pp.tile([P, DH], F32, name=f"pv_{blk}_{sc}", tag="sc", bufs=2)
                for dc in range(NDC):
                    nc.tensor.matmul(pv[:], xt[:, ds(dc * SB + sc * P, P)],
                                     wv_sb[:, ds(dc * DH, DH)],
                                     start=(dc == 0), stop=(dc == NDC - 1))
                if sc == 0:
                    qbs["q1"] = chain_act("q1")
                nc.vector.tensor_copy(v_sb[blk * (SB // P) + sc][:], pv[:])
            late = ("q1",) if blk == 0 else ("k", "q0", "q1")
            for e in late:
                emit_ropemuls(e, qbs[e], info[e][0], info[e][1], *dests[e])

        def p3(blk):
            """o-projection for block blk -> DRAM. All copies on Act: its
            queue is clear here (norm batch finished during V), while DVE
            holds the rope batch. DMA per 4-op group keeps HWDGE slots few."""
            sblice = ds(blk * SB, SB)
            finegrain = blk == NSB - 1  # last block: per-op DMA, short drain
            for g4 in range(4):
                ob4 = sp.tile([P, 4 * SB], BF16, name=f"ob_{blk}_{g4}", tag="ob", bufs=2)
                for j in range(4):
                    dmc = 4 * g4 + j
                    op = pp.tile([P, SB], F32, name=f"op_{blk}_{dmc}", tag="pj", bufs=4)
                    for ec in range(4):
                        nc.tensor.matmul(op[:], wo_sb[:, ds(ec * D + dmc * P, P)],
                                         ao_sb[ec][:, sblice],
                                         start=(ec == 0), stop=(ec == 3))
                    if finegrain and g4 == 3 and j == 3:
                        nc.vector.tensor_copy(ob4[:, ds(j * SB, SB)], op[:])
                    else:
                        nc.scalar.copy(ob4[:, ds(j * SB, SB)], op[:])
                    if finegrain:
                        nc.sync.dma_start(
                            out_r[:, ds(dmc, 1), sblice],
                            ob4[:, ds(j * SB, SB)].rearrange("p (g s) -> p g s", g=1))
                if not finegrain:
                    nc.sync.dma_start(
                        out_r[:, ds(4 * g4, 4), sblice],
                        ob4[:].rearrange("p (g s) -> p g s", g=4))

        def p2(blk):
            """Attention for the two q-tiles of block blk.

            Chunk emission order sandwiches the half-width (trimmed) chunks
            between full ones, so the PV psum accumulation starts on the
            first full chunk (full width) and stops on the last full chunk,
            with the trims accumulating into column sub-ranges in between.
            q0=0 has a single full chunk, so its diagonal trim keeps the
            legacy zero-half pth tile and accumulates at full width.
            """
            for it, (sub, h) in enumerate([(s_, h_) for h_ in range(2) for s_ in range(2)]):
                q0 = blk * SB + sub * QB
                raw = _kchunks(q0)
                fulls = [c for c in raw if c[3] == QB]
                trims = [c for c in raw if c[3] != QB]
                if len(fulls) >= 2:
                    chunks = fulls[:-1] + trims + [fulls[-1]]
                else:
                    # q-tile 0: single full chunk, so no room to sandwich the
                    # diagonal trim -- widen it to a full mask-1 chunk instead
                    chunks = fulls + [(kc, 1, 0, QB) for (kc, _, _, _) in trims]
                nch = len(chunks)
                # pack chunks into score/exp groups of <= SB columns so one
                # [P, <=512] exp serves several chunks (Act paces attention)
                groups = []  # per chunk: (gid, col offset); plus per-group width
                gw, gid = 0, 0
                gwidths = []
                for c in chunks:
                    if gw + c[3] > SB:
                        gwidths.append(gw)
                        gid, gw = gid + 1, 0
                    groups.append((gid, gw))
                    gw += c[3]
                gwidths.append(gw)
                glast = [i for i in range(nch)
                         if i + 1 == nch or groups[i + 1][0] != groups[i][0]]
                ao0 = pp.tile([P, QB], F32, name=f"ao0_{q0}_{h}", tag="ao", bufs=2)
                ao1 = pp.tile([P, QB], F32, name=f"ao1_{q0}_{h}", tag="ao", bufs=2)
                pacc = sp.tile([P, QB], F32, name=f"pacc_{q0}_{h}", tag="pacc", bufs=2)
                pts = [None] * nch
                scps = {}
                for i in range(nch + LOOK):
                    if i < nch:
                        kc, mi, qoff, qw = chunks[i]
                        gid, co = groups[i]
                        if gid not in scps:
                            scp = pp.tile([P, SB], F32, name=f"sc_{q0}_{h}_{gid}",
                                          tag="sc", bufs=2)
                            ptg = sp.tile([P, SB], BF16, name=f"pt_{q0}_{h}_{gid}",
                                          tag="pt", bufs=6)
                            scps[gid] = (scp, ptg)
                        scp, ptg = scps[gid]
                        qsl = ds(q0 + qoff, qw)
                        csl = ds(co, qw)
                        nc.tensor.matmul(scp[:, csl], k_sb[0][:, ts(kc, P)],
                                         q_sb[2 * h][:, qsl], start=True, stop=False)
                        nc.tensor.matmul(scp[:, csl], k_sb[1][:, ts(kc, P)],
                                         q_sb[2 * h + 1][:, qsl], start=False, stop=True)
                        pts[i] = (ptg, co)
                        if i in glast:
                            w = gwidths[gid]
                            nc.scalar.activation(ptg[:, 0:w], scp[:, 0:w], AF.Exp)
                            # masks + denominator adds for the group's chunks
                            for i2 in range(nch):
                                if groups[i2][0] != gid:
                                    continue
                                kc2, mi2, qoff2, qw2 = chunks[i2]
                                co2 = groups[i2][1]
                                live = ptg[:, ds(co2, qw2)]
                                if mi2 is not None:
                                    nc.vector.tensor_mul(
                                        live, live, msk_sb[:, ds(mi2 * SB, qw2)])
                                pql = ds(qoff2, qw2)
                                if i2 == 0:
                                    nc.vector.tensor_copy(pacc[:, pql], live)
                                else:
                                    nc.vector.tensor_add(pacc[:, pql],
                                                         pacc[:, pql], live)
                    j = i - LOOK
                    if 0 <= j < nch:
                        kc, mi, qoff, qw = chunks[j]
                        ptg, co = pts[j]
                        first, last = j == 0, j == nch - 1
                        osl, psl = ds(qoff, qw), ds(co, qw)
                        nc.tensor.matmul(ao0[:, osl], v_sb[kc][:, 0:P], ptg[:, psl],
                                         start=first, stop=last)
                        nc.tensor.matmul(ao1[:, osl], v_sb[kc][:, P:DH], ptg[:, psl],
                                         start=first, stop=last)
                dsum = sp.tile([P, QB], F32, name=f"ds_{q0}_{h}", tag="dsum", bufs=2)
                nc.gpsimd.partition_all_reduce(dsum[:], pacc[:], P,
                                               bass_isa.ReduceOp.add)
                db = sp.tile([P, QB], F32, name=f"db_{q0}_{h}", tag="db", bufs=2)
                nc.vector.reciprocal(db[:], dsum[:])
                qslice = ds(q0, QB)
                nc.vector.tensor_mul(ao_sb[2 * h][:, qslice], ao0[:], db[:])
                nc.vector.tensor_mul(ao_sb[2 * h + 1][:, qslice], ao1[:], db[:])

        # =================== main schedule ===================
        for blk in range(NSB):
            p1(blk)
            if blk == 0:
                nc.sync.dma_start(msk_sb[:].rearrange("p (m j) -> p m j", m=8),
                                  masks.rearrange("m p j -> p m j"))
                nc.sync.dma_start(wo_sb[:].rearrange("p (c e) -> p c e", c=4),
                                  woT.rearrange("(c p) e -> p c e", p=P))
            if blk > 0:
                p3(blk - 1)
            # prefetch next block's x/trig AFTER p3's out-DMAs are queued,
            # so they drain during attention when the DMA engines are idle
            if blk + 1 < NSB:
                xt = sp.tile([P, NDC * SB], BF16, name=f"xt{blk+1}", tag="xt", bufs=2)
                for g in range(2):
                    nc.sync.dma_start(
                        xt[:, ds(g * 8 * SB, 8 * SB)].rearrange("p (c s) -> p c s", c=8),
                        xT_r[:, ds(8 * g, 8), ds((blk + 1) * SB, SB)])
                xts[blk + 1] = xt
                tgt = sp.tile([P, 2 * SB], BF16, name=f"tg{blk+1}", tag="tg", bufs=2)
                nc.sync.dma_start(tgt[:].rearrange("p (r s) -> p r s", r=2),
                                  trig.rearrange("r p s -> p r s")[:, :, ds((blk + 1) * SB, SB)])
                tgs[blk + 1] = tgt
            p2(blk)
        p3(NSB - 1)

    # Pin the activation-table pass to the one set containing ln+exp+copy
    # (canonical set ids preserved; other sets hidden so the per-block
    # ln/exp sequence never reloads tables). Restored right after compile.
    import concourse.bacc as _bacc_mod

    orig_gat = _bacc_mod.get_activation_tables

    def pinned_gat(arch):
        return {name: (funcs if name == "natural_log_exp_and_others" else set())
                for name, funcs in orig_gat(arch).items()}

    _bacc_mod.get_activation_tables = pinned_gat
    try:
        nc.compile()
    finally:
        _bacc_mod.get_activation_tables = orig_gat
    return nc


_NC = None
LAST_RESULT = None


def _get_nc():
    global _NC
    if _NC is None:
        _NC = _build()
    return _NC


def _host_tables(q_norm_w, k_norm_w):
    qw, kw = np.asarray(q_norm_w, np.float64), np.asarray(k_norm_w, np.float64)
    # device shares one cos/sin table across q/k and both rotary halves;
    # requires uniform (1 + w) factors (true for Gemma-zero-init norm weights)
    assert np.allclose(qw, qw[0]) and np.allclose(kw, kw[0]) and np.allclose(qw[0], kw[0]), \
        "non-uniform q/k norm weights need the 8-row trig layout"
    c = 1.0 + qw[0]
    inv_freq = 1.0 / (ROPE_BASE ** (np.arange(0, DH, 2, dtype=np.float64) / DH))
    freqs = np.outer(np.arange(S, dtype=np.float64), inv_freq)   # [S, DH/2]
    cos = (np.cos(freqs) * c).T.astype(np.float32)               # [DH/2, S]
    sin = (np.sin(freqs) * c).T.astype(np.float32)
    trig = np.stack([cos, sin]).astype(NPBF16)                   # [2, 128, S]

    i = np.arange(P)[:, None]
    j = np.arange(SB)[None, :]
    mrows = [(j >= i + P * o) for o in range(4)] + [(j <= i + P * o - 1) for o in range(4)]
    masks = np.stack(mrows).astype(NPBF16)
    ones = np.ones((P, 1), NPBF16)
    return trig, masks, ones


def kernel(hidden_states, Wq, Wk, Wv, Wo, q_norm_w, k_norm_w):
    global LAST_RESULT
    nc = _get_nc()
    trig, masks, ones = _host_tables(np.asarray(q_norm_w), np.asarray(k_norm_w))

    xTs = [np.ascontiguousarray(np.asarray(hidden_states)[b].T).astype(NPBF16)
           for b in range(B)]
    in_maps = []
    for core in range(8):
        b, g = core // 4, core % 4
        in_maps.append({
            "xT": xTs[b],
            "wqT": np.ascontiguousarray(np.asarray(Wq)[g * EQ:(g + 1) * EQ, :].T).astype(NPBF16),
            "wkT": np.ascontiguousarray(np.asarray(Wk)[g * DH:(g + 1) * DH, :].T).astype(NPBF16),
            "wvT": np.ascontiguousarray(np.asarray(Wv)[g * DH:(g + 1) * DH, :].T).astype(NPBF16),
            "woT": np.ascontiguousarray(np.asarray(Wo)[:, g * EQ:(g + 1) * EQ].T).astype(NPBF16),
            "trig": trig,
            "masks": masks,
            "onesd": ones,
        })

    LAST_RESULT = run_bass_kernel_spmd(nc, in_maps, list(range(8)))
    res = LAST_RESULT.results
    outs = []
    for b in range(B):
        acc = np.zeros((D, S), np.float32)
        for g in range(4):
            acc += res[4 * b + g]["out"].astype(np.float32)
        outs.append(acc.T)
    return np.stack(outs).astype(np.float32)


# revision 76
# speedup vs baseline: 1.0114x; 1.0052x over previous
"""Gemma sliding-window attention layer on 8 Trainium2 NeuronCores.

Sharding: data-parallel over batch (B=2) x tensor-parallel over heads
(4 groups: 2 q heads + 1 kv head each) = 8 cores. Each core computes a
partial o-proj output [D, S] in bf16; host sums the 4 TP partials per
batch in fp32 and transposes back to [S, D].

Schedule (per 512-wide seq block b): proj(b) -> o-proj(b-1) -> attn(b),
so the rmsnorm/rope chain of block b and the softmax normalize of block
b-1 hide behind independent PE work. Within attention, score matmuls
run a fixed lookahead ahead of the PV matmuls so exp/mask latency never
stalls the PE. RMSNorm rinv = exp(-0.5*ln(ms)) keeps the activation
engine on a single function table (ln+exp share one set; sqrt doesn't).
RoPE runs in bf16 on SBUF for the DVE 2x mode. Half-width diagonal /
window-edge score tiles write into persistent half-zeroed pt tiles so
the PV accumulation stays full-width.
"""

import sys

sys.path.insert(0, "/opt/trn_rl_repo")

import math
from contextlib import ExitStack

import numpy as np
import ml_dtypes

import concourse.bass as bass
import concourse.bass_isa as bass_isa
import concourse.tile as tile
from concourse import bacc, mybir
from concourse.bass import ds, ts
from concourse.bass_utils import run_bass_kernel_spmd

BF16 = mybir.dt.bfloat16
F32 = mybir.dt.float32
NPBF16 = ml_dtypes.bfloat16
AF = mybir.ActivationFunctionType

H, KVH, DH, SW = 8, 4, 256, 1024
B, S, D = 2, 2048, 2048
EPS = 1e-6
ROPE_BASE = 10000.0
P = 128
SB = 512          # s-block width
NSB = S // SB     # 4
NDC = D // P      # 16 contraction chunks
EQ = 2 * DH       # per-core q width (2 heads)
QB = 256          # attention q-tile width
KBIAS = -math.log(16.0)  # folds the 1/sqrt(256) attn scale into k's rinv
LOOK = 3          # score-tile lookahead in the attention pipeline


def _kchunks(q0):
    """k-chunks for q-tile [q0, q0+QB).

    Returns (kc, mi, qoff, qw): k-chunk index, mask index or None, live
    q-column offset/width.  pT tile [k=128 (i), q (j)]: kpos = 128*kc + i,
    qpos = q0 + qoff + j; valid iff 0 <= qpos - kpos < SW.
    Masks: 0 -> j >= i (causal diag), 4 -> j <= i-1, 5 -> j <= i+127.
    d = 128*kc - q0:  d = 0 full mask0; d = +128 half tile (qoff 128,
    mask0 cols 0:128); d + SW = 0 half tile (qoff 0, mask4 cols 0:128);
    d + SW = 128 full tile mask5; other in-range chunks are mask-free.
    """
    res = []
    for KC in range(max(0, q0 - SW), q0 + QB, P):
        d = KC - q0
        if d == P:
            res.append((KC // P, 0, P, P))
        elif d == 0:
            res.append((KC // P, 0, 0, QB))
        elif d + SW == 0:
            res.append((KC // P, 4, 0, P))
        elif d + SW == P:
            res.append((KC // P, 5, 0, QB))
        else:
            res.append((KC // P, None, 0, QB))
    return res


def _build():
    nc = bacc.Bacc("TRN2", target_bir_lowering=False, debug=False)

    xT = nc.dram_tensor("xT", [D, S], BF16, kind="ExternalInput")
    wqT = nc.dram_tensor("wqT", [D, EQ], BF16, kind="ExternalInput")
    wkT = nc.dram_tensor("wkT", [D, DH], BF16, kind="ExternalInput")
    wvT = nc.dram_tensor("wvT", [D, DH], BF16, kind="ExternalInput")
    woT = nc.dram_tensor("woT", [EQ, D], BF16, kind="ExternalInput")
    trig = nc.dram_tensor("trig", [2, P, S], BF16, kind="ExternalInput")  # cos, sin
    masks = nc.dram_tensor("masks", [8, P, SB], BF16, kind="ExternalInput")
    onesd = nc.dram_tensor("onesd", [P, 1], BF16, kind="ExternalInput")
    out = nc.dram_tensor("out", [D, S], BF16, kind="ExternalOutput")

    # const tiles for activation bias operands
    for val in (EPS, KBIAS):
        t = nc.alloc_sbuf_tensor(f"const-bias-{val}", [P, 1], F32)
        nc.gpsimd.memset(t.ap(), val)
        nc.const_aps.aps[(F32, val)] = t.ap()
    nc.all_engine_barrier()

    with tile.TileContext(nc) as tc, ExitStack() as ctx:
        sp = ctx.enter_context(tc.tile_pool(name="sp", bufs=2))    # SBUF
        pp = ctx.enter_context(tc.tile_pool(name="pp", bufs=2, space="PSUM"))

        # ---- persistent SBUF tiles ----
        wq_sb = sp.tile([P, NDC * EQ], BF16, name="wq", tag="wq", bufs=1)
        wk_sb = sp.tile([P, NDC * DH], BF16, name="wk", tag="wk", bufs=1)
        wv_sb = sp.tile([P, NDC * DH], BF16, name="wv", tag="wv", bufs=1)
        wo_sb = sp.tile([P, 4 * D], BF16, name="wo", tag="wo", bufs=1)
        msk_sb = sp.tile([P, 8 * SB], BF16, name="msk", tag="msk", bufs=1)
        q_sb = [sp.tile([P, S], BF16, name=f"qsb{i}", tag="qsb", bufs=4) for i in range(4)]
        k_sb = [sp.tile([P, S], BF16, name=f"ksb{i}", tag="ksb", bufs=2) for i in range(2)]
        v_sb = [sp.tile([P, DH], BF16, name=f"vsb{i}", tag="vsb", bufs=NDC) for i in range(NDC)]
        ao_sb = [sp.tile([P, S], BF16, name=f"aosb{i}", tag="aosb", bufs=4) for i in range(4)]


        # ---- startup loads: k-ent weights + x block 0 first, fine-grained ----
        xT_r = xT.rearrange("(c p) s -> p c s", p=P)
        wk_r = wkT.rearrange("(c p) e -> p c e", p=P)
        wv_r = wvT.rearrange("(c p) e -> p c e", p=P)
        wq_r = wqT.rearrange("(c p) e -> p c e", p=P)

        xt0 = sp.tile([P, NDC * SB], BF16, name="xt0", tag="xt", bufs=2)
        nc.sync.dma_start(
            wk_sb[:, ds(0, 4 * DH)].rearrange("p (c e) -> p c e", c=4),
            wk_r[:, ds(0, 4), :])
        nc.sync.dma_start(
            xt0[:, ds(0, SB)].rearrange("p (c s) -> p c s", c=1),
            xT_r[:, ds(0, 1), ds(0, SB)])
        nc.sync.dma_start(
            xt0[:, ds(SB, SB)].rearrange("p (c s) -> p c s", c=1),
            xT_r[:, ds(1, 1), ds(0, SB)])
        for g in range(1, 8):
            nc.sync.dma_start(
                xt0[:, ds(g * 2 * SB, 2 * SB)].rearrange("p (c s) -> p c s", c=2),
                xT_r[:, ds(2 * g, 2), ds(0, SB)])
            if g in (1, 3, 5):
                gk = (g + 1) // 2
                nc.sync.dma_start(
                    wk_sb[:, ds(gk * 4 * DH, 4 * DH)].rearrange("p (c e) -> p c e", c=4),
                    wk_r[:, ds(4 * gk, 4), :])
        for g in range(4):
            nc.sync.dma_start(
                wq_sb[:, ds(g * 4 * EQ, 4 * EQ)].rearrange("p (c e) -> p c e", c=4),
                wq_r[:, ds(4 * g, 4), :])
        for g in range(2):
            nc.sync.dma_start(
                wv_sb[:, ds(g * 8 * DH, 8 * DH)].rearrange("p (c e) -> p c e", c=8),
                wv_r[:, ds(8 * g, 8), :])
        tg0 = sp.tile([P, 2 * SB], BF16, name="tg0", tag="tg", bufs=2)
        nc.sync.dma_start(tg0[:].rearrange("p (r s) -> p r s", r=2),
                          trig.rearrange("r p s -> p r s")[:, :, ds(0, SB)])

        def wq_ap(dc, eoff):
            return wq_sb[:, ds(dc * EQ + eoff, P)]

        def wk_ap(dc, eoff):
            return wk_sb[:, ds(dc * DH + eoff, P)]

        out_r = out.rearrange("(g p) s -> p g s", p=P)

        xts = {0: xt0}
        tgs = {0: tg0}

        # =================== phase emitters ===================

        def p1(blk):
            """Projections + rmsnorm + rope for seq block blk (k, V, q0, q1)."""
            xt = xts.pop(blk)
            tgt = tgs.pop(blk)
            sblice = ds(blk * SB, SB)
            tcos, tsin = tgt[:, 0:SB], tgt[:, SB : 2 * SB]

            ents = [("k", wk_ap, 0, 0), ("q0", wq_ap, 0, 32), ("q1", wq_ap, DH, 64)]

            def proj(ent, w_ap, eoff):
                pa = pp.tile([P, SB], F32, name=f"pa_{blk}_{ent}", tag="pj", bufs=3)
                pb = pp.tile([P, SB], F32, name=f"pb_{blk}_{ent}", tag="pj", bufs=3)
                for dc in range(NDC):
                    nc.tensor.matmul(pa[:], w_ap(dc, eoff), xt[:, ds(dc * SB, SB)],
                                     start=(dc == 0), stop=(dc == NDC - 1))
                pab = sp.tile([P, SB], BF16, name=f"pab_{blk}_{ent}", tag="pr", bufs=6)
                nc.vector.tensor_copy(pab[:], pa[:])
                for dc in range(NDC):
                    nc.tensor.matmul(pb[:], w_ap(dc, eoff + P), xt[:, ds(dc * SB, SB)],
                                     start=(dc == 0), stop=(dc == NDC - 1))
                pbb = sp.tile([P, SB], BF16, name=f"pbb_{blk}_{ent}", tag="pr", bufs=6)
                nc.vector.tensor_copy(pbb[:], pb[:])
                sqa = sp.tile([P, SB], BF16, name=f"sqa_{blk}_{ent}", tag="sq", bufs=4)
                sqb = sp.tile([P, SB], BF16, name=f"sqb_{blk}_{ent}", tag="sq", bufs=4)
                nc.vector.tensor_mul(sqa[:], pab[:], pab[:])
                nc.vector.tensor_mul(sqb[:], pbb[:], pbb[:])
                return pab, pbb, sqa, sqb

            def emit_sums(ent, sqa, sqb):
                # partition sums on the (otherwise idle) gpsimd engine; the
                # all-reduce result arrives already broadcast over partitions,
                # so ln/exp produce the full [P, SB] rinv with no bcast step.
                ra = sp.tile([P, SB], F32, name=f"ra_{blk}_{ent}", tag="ra", bufs=2)
                rb = sp.tile([P, SB], F32, name=f"rb_{blk}_{ent}", tag="ra", bufs=2)
                nc.gpsimd.partition_all_reduce(ra[:], sqa[:], P, bass_isa.ReduceOp.add)
                nc.gpsimd.partition_all_reduce(rb[:], sqb[:], P, bass_isa.ReduceOp.add)
                rs = sp.tile([P, SB], F32, name=f"rs_{blk}_{ent}", tag="rs", bufs=3)
                nc.gpsimd.tensor_add(rs[:], ra[:], rb[:])
                return rs

            def emit_ln(ent, rs):
                tln = sp.tile([P, SB], F32, name=f"tln_{blk}_{ent}", tag="tl", bufs=3)
                nc.scalar.activation(tln[:], rs[:], AF.Ln, bias=EPS, scale=1.0 / DH)
                return tln

            def emit_exp(ent, tln):
                qb = sp.tile([P, SB], BF16, name=f"qb_{blk}_{ent}", tag="qb", bufs=4)
                nc.scalar.activation(qb[:], tln[:], AF.Exp, scale=-0.5,
                                     bias=(KBIAS if ent == "k" else 0.0))
                return qb

            def emit_ropemuls(ent, qb, pab, pbb, lo, hi):
                ta = sp.tile([P, SB], BF16, name=f"ta_{blk}_{ent}", tag="rt", bufs=8)
                tb = sp.tile([P, SB], BF16, name=f"tb_{blk}_{ent}", tag="rt", bufs=8)
                tc_ = sp.tile([P, SB], BF16, name=f"tc_{blk}_{ent}", tag="rt", bufs=8)
                nc.vector.tensor_mul(ta[:], pab[:], tcos)
                nc.vector.tensor_mul(tb[:], pbb[:], tsin)
                nc.vector.tensor_sub(tc_[:], ta[:], tb[:])
                nc.vector.tensor_mul(lo[:, sblice], tc_[:], qb[:])
                td = sp.tile([P, SB], BF16, name=f"td_{blk}_{ent}", tag="rt", bufs=8)
                te = sp.tile([P, SB], BF16, name=f"te_{blk}_{ent}", tag="rt", bufs=8)
                tf = sp.tile([P, SB], BF16, name=f"tf_{blk}_{ent}", tag="rt", bufs=8)
                nc.vector.tensor_mul(td[:], pbb[:], tcos)
                nc.vector.tensor_mul(te[:], pab[:], tsin)
                nc.vector.tensor_add(tf[:], td[:], te[:])
                nc.vector.tensor_mul(hi[:, sblice], tf[:], qb[:])

            dests = {"k": (k_sb[0], k_sb[1]), "q0": (q_sb[0], q_sb[1]),
                     "q1": (q_sb[2], q_sb[3])}
            info = {}

            def chain_act(e):
                return emit_exp(e, emit_ln(e, emit_sums(e, info[e][2], info[e][3])))

            # --- k, q0, q1 projections, then V. Each ent's sums/ln/exp chain
            # goes out as soon as its squares exist (no act-table switches:
            # the table set is pinned, see compile below). Rope muls (DVE) are
            # emitted after the V copies so the pv/op psum rings never wait
            # behind them in the DVE queue; block 0 (no o-proj cover ahead of
            # attention) emits k/q0 ropes early instead.
            qbs = {}
            info["k"] = proj("k", wk_ap, 0)
            info["q0"] = proj("q0", wq_ap, 0)
            qbs["k"] = chain_act("k")
            if blk == 0:
                emit_ropemuls("k", qbs["k"], info["k"][0], info["k"][1], *dests["k"])
            info["q1"] = proj("q1", wq_ap, DH)
            qbs["q0"] = chain_act("q0")
            if blk == 0:
                emit_ropemuls("q0", qbs["q0"], info["q0"][0], info["q0"][1],
                              *dests["q0"])

            for sc in range(SB // P):
                pv = pp.tile([P, DH], F32, name=f"pv_{blk}_{sc}", tag="sc", bufs=3)
                for dc in range(NDC):
                    nc.tensor.matmul(pv[:], xt[:, ds(dc * SB + sc * P, P)],
                                     wv_sb[:, ds(dc * DH, DH)],
                                     start=(dc == 0), stop=(dc == NDC - 1))
                if sc == 0:
                    qbs["q1"] = chain_act("q1")
                nc.vector.tensor_copy(v_sb[blk * (SB // P) + sc][:], pv[:])
            late = ("q1",) if blk == 0 else ("k", "q0", "q1")
            for e in late:
                emit_ropemuls(e, qbs[e], info[e][0], info[e][1], *dests[e])

        def p3(blk):
            """o-projection for block blk -> DRAM. All copies on Act: its
            queue is clear here (norm batch finished during V), while DVE
            holds the rope batch. DMA per 4-op group keeps HWDGE slots few."""
            sblice = ds(blk * SB, SB)
            finegrain = blk == NSB - 1  # last block: per-op DMA, short drain
            for g4 in range(4):
                ob4 = sp.tile([P, 4 * SB], BF16, name=f"ob_{blk}_{g4}", tag="ob", bufs=2)
                for j in range(4):
                    dmc = 4 * g4 + j
                    op = pp.tile([P, SB], F32, name=f"op_{blk}_{dmc}", tag="pj", bufs=3)
                    for ec in range(4):
                        nc.tensor.matmul(op[:], wo_sb[:, ds(ec * D + dmc * P, P)],
                                         ao_sb[ec][:, sblice],
                                         start=(ec == 0), stop=(ec == 3))
                    if finegrain and g4 == 3 and j == 3:
                        nc.vector.tensor_copy(ob4[:, ds(j * SB, SB)], op[:])
                    else:
                        nc.scalar.copy(ob4[:, ds(j * SB, SB)], op[:])
                    if finegrain:
                        nc.sync.dma_start(
                            out_r[:, ds(dmc, 1), sblice],
                            ob4[:, ds(j * SB, SB)].rearrange("p (g s) -> p g s", g=1))
                if not finegrain:
                    nc.sync.dma_start(
                        out_r[:, ds(4 * g4, 4), sblice],
                        ob4[:].rearrange("p (g s) -> p g s", g=4))

        def p2(blk):
            """Attention for the two q-tiles of block blk.

            Chunk emission order sandwiches the half-width (trimmed) chunks
            between full ones, so the PV psum accumulation starts on the
            first full chunk (full width) and stops on the last full chunk,
            with the trims accumulating into column sub-ranges in between.
            q0=0 has a single full chunk, so its diagonal trim keeps the
            legacy zero-half pth tile and accumulates at full width.
            """
            for it, (sub, h) in enumerate([(s_, h_) for h_ in range(2) for s_ in range(2)]):
                q0 = blk * SB + sub * QB
                raw = _kchunks(q0)
                fulls = [c for c in raw if c[3] == QB]
                trims = [c for c in raw if c[3] != QB]
                if len(fulls) >= 2:
                    chunks = fulls[:-1] + trims + [fulls[-1]]
                else:
                    # q-tile 0: single full chunk, so no room to sandwich the
                    # diagonal trim -- widen it to a full mask-1 chunk instead
                    chunks = fulls + [(kc, 1, 0, QB) for (kc, _, _, _) in trims]
                nch = len(chunks)
                # pack chunks into score/exp groups of <= SB columns so one
                # [P, <=512] exp serves several chunks (Act paces attention)
                groups = []  # per chunk: (gid, col offset); plus per-group width
                gw, gid = 0, 0
                gwidths = []
                for c in chunks:
                    if gw + c[3] > SB:
                        gwidths.append(gw)
                        gid, gw = gid + 1, 0
                    groups.append((gid, gw))
                    gw += c[3]
                gwidths.append(gw)
                glast = [i for i in range(nch)
                         if i + 1 == nch or groups[i + 1][0] != groups[i][0]]
                ao0 = pp.tile([P, QB], F32, name=f"ao0_{q0}_{h}", tag="ao", bufs=2)
                ao1 = pp.tile([P, QB], F32, name=f"ao1_{q0}_{h}", tag="ao", bufs=2)
                pacc = sp.tile([P, QB], F32, name=f"pacc_{q0}_{h}", tag="pacc", bufs=2)
                pts = [None] * nch
                scps = {}
                for i in range(nch + LOOK):
                    if i < nch:
                        kc, mi, qoff, qw = chunks[i]
                        gid, co = groups[i]
                        if gid not in scps:
                            scp = pp.tile([P, SB], F32, name=f"sc_{q0}_{h}_{gid}",
                                          tag="sc", bufs=3)
                            ptg = sp.tile([P, SB], BF16, name=f"pt_{q0}_{h}_{gid}",
                                          tag="pt", bufs=6)
                            scps[gid] = (scp, ptg)
                        scp, ptg = scps[gid]
                        qsl = ds(q0 + qoff, qw)
                        csl = ds(co, qw)
                        nc.tensor.matmul(scp[:, csl], k_sb[0][:, ts(kc, P)],
                                         q_sb[2 * h][:, qsl], start=True, stop=False)
                        nc.tensor.matmul(scp[:, csl], k_sb[1][:, ts(kc, P)],
                                         q_sb[2 * h + 1][:, qsl], start=False, stop=True)
                        pts[i] = (ptg, co)
                        if i in glast:
                            w = gwidths[gid]
                            nc.scalar.activation(ptg[:, 0:w], scp[:, 0:w], AF.Exp)
                            # masks + denominator adds for the group's chunks
                            for i2 in range(nch):
                                if groups[i2][0] != gid:
                                    continue
                                kc2, mi2, qoff2, qw2 = chunks[i2]
                                co2 = groups[i2][1]
                                live = ptg[:, ds(co2, qw2)]
                                if mi2 is not None:
                                    nc.vector.tensor_mul(
                                        live, live, msk_sb[:, ds(mi2 * SB, qw2)])
                                pql = ds(qoff2, qw2)
                                if i2 == 0:
                                    nc.vector.tensor_copy(pacc[:, pql], live)
                                else:
                                    nc.vector.tensor_add(pacc[:, pql],
                                                         pacc[:, pql], live)
                    j = i - LOOK
                    if 0 <= j < nch:
                        kc, mi, qoff, qw = chunks[j]
                        ptg, co = pts[j]
                        first, last = j == 0, j == nch - 1
                        osl, psl = ds(qoff, qw), ds(co, qw)
                        nc.tensor.matmul(ao0[:, osl], v_sb[kc][:, 0:P], ptg[:, psl],
                                         start=first, stop=last)
                        nc.tensor.matmul(ao1[:, osl], v_sb[kc][:, P:DH], ptg[:, psl],
                                         start=first, stop=last)
                dsum = sp.tile([P, QB], F32, name=f"ds_{q0}_{h}", tag="dsum", bufs=2)
                nc.gpsimd.partition_all_reduce(dsum[:], pacc[:], P,
                                               bass_isa.ReduceOp.add)
                db = sp.tile([P, QB], F32, name=f"db_{q0}_{h}", tag="db", bufs=2)
                nc.vector.reciprocal(db[:], dsum[:])
                qslice = ds(q0, QB)
                nc.vector.tensor_mul(ao_sb[2 * h][:, qslice], ao0[:], db[:])
                nc.vector.tensor_mul(ao_sb[2 * h + 1][:, qslice], ao1[:], db[:])

        # =================== main schedule ===================
        for blk in range(NSB):
            p1(blk)
            if blk == 0:
                nc.sync.dma_start(msk_sb[:].rearrange("p (m j) -> p m j", m=8),
                                  masks.rearrange("m p j -> p m j"))
                nc.sync.dma_start(wo_sb[:].rearrange("p (c e) -> p c e", c=4),
                                  woT.rearrange("(c p) e -> p c e", p=P))
            if blk > 0:
                p3(blk - 1)
            # prefetch next block's x/trig AFTER p3's out-DMAs are queued,
            # so they drain during attention when the DMA engines are idle
            if blk + 1 < NSB:
                xt = sp.tile([P, NDC * SB], BF16, name=f"xt{blk+1}", tag="xt", bufs=2)
                for g in range(2):
                    nc.sync.dma_start(
                        xt[:, ds(g * 8 * SB, 8 * SB)].rearrange("p (c s) -> p c s", c=8),
                        xT_r[:, ds(8 * g, 8), ds((blk + 1) * SB, SB)])
                xts[blk + 1] = xt
                tgt = sp.tile([P, 2 * SB], BF16, name=f"tg{blk+1}", tag="tg", bufs=2)
                nc.sync.dma_start(tgt[:].rearrange("p (r s) -> p r s", r=2),
                                  trig.rearrange("r p s -> p r s")[:, :, ds((blk + 1) * SB, SB)])
                tgs[blk + 1] = tgt
            p2(blk)
        p3(NSB - 1)

    # Pin the activation-table pass to the one set containing ln+exp+copy
    # (canonical set ids preserved; other sets hidden so the per-block
    # ln/exp sequence never reloads tables). Restored right after compile.
    import concourse.bacc as _bacc_mod

    orig_gat = _bacc_mod.get_activation_tables

    def pinned_gat(arch):
        return {name: (funcs if name == "natural_log_exp_and_others" else set())
                for name, funcs in orig_gat(arch).items()}

    _bacc_mod.get_activation_tables = pinned_gat
    try:
        nc.compile()
    finally:
        _bacc_mod.get_activation_tables = orig_gat
    return nc


_NC = None
LAST_RESULT = None


def _get_nc():
    global _NC
    if _NC is None:
        _NC = _build()
    return _NC


def _host_tables(q_norm_w, k_norm_w):
    qw, kw = np.asarray(q_norm_w, np.float64), np.asarray(k_norm_w, np.float64)
    # device shares one cos/sin table across q/k and both rotary halves;
    # requires uniform (1 + w) factors (true for Gemma-zero-init norm weights)
    assert np.allclose(qw, qw[0]) and np.allclose(kw, kw[0]) and np.allclose(qw[0], kw[0]), \
        "non-uniform q/k norm weights need the 8-row trig layout"
    c = 1.0 + qw[0]
    inv_freq = 1.0 / (ROPE_BASE ** (np.arange(0, DH, 2, dtype=np.float64) / DH))
    freqs = np.outer(np.arange(S, dtype=np.float64), inv_freq)   # [S, DH/2]
    cos = (np.cos(freqs) * c).T.astype(np.float32)               # [DH/2, S]
    sin = (np.sin(freqs) * c).T.astype(np.float32)
    trig = np.stack([cos, sin]).astype(NPBF16)                   # [2, 128, S]

    i = np.arange(P)[:, None]
    j = np.arange(SB)[None, :]
    mrows = [(j >= i + P * o) for o in range(4)] + [(j <= i + P * o - 1) for o in range(4)]
    masks = np.stack(mrows).astype(NPBF16)
    ones = np.ones((P, 1), NPBF16)
    return trig, masks, ones


def kernel(hidden_states, Wq, Wk, Wv, Wo, q_norm_w, k_norm_w):
    global LAST_RESULT
    nc = _get_nc()
    trig, masks, ones = _host_tables(np.asarray(q_norm_w), np.asarray(k_norm_w))

    xTs = [np.ascontiguousarray(np.asarray(hidden_states)[b].T).astype(NPBF16)
           for b in range(B)]
    in_maps = []
    for core in range(8):
        b, g = core // 4, core % 4
        in_maps.append({
            "xT": xTs[b],
            "wqT": np.ascontiguousarray(np.asarray(Wq)[g * EQ:(g + 1) * EQ, :].T).astype(NPBF16),
            "wkT": np.ascontiguousarray(np.asarray(Wk)[g * DH:(g + 1) * DH, :].T).astype(NPBF16),
            "wvT": np.ascontiguousarray(np.asarray(Wv)[g * DH:(g + 1) * DH, :].T).astype(NPBF16),
            "woT": np.ascontiguousarray(np.asarray(Wo)[:, g * EQ:(g + 1) * EQ].T).astype(NPBF16),
            "trig": trig,
            "masks": masks,
            "onesd": ones,
        })

    LAST_RESULT = run_bass_kernel_spmd(nc, in_maps, list(range(8)))
    res = LAST_RESULT.results
    outs = []
    for b in range(B):
        acc = np.zeros((D, S), np.float32)
        for g in range(4):
            acc += res[4 * b + g]["out"].astype(np.float32)
        outs.append(acc.T)
    return np.stack(outs).astype(np.float32)
